# revision 1
# baseline (speedup 1.0000x reference)
"""Butterfly-Conv2d (nn_BConv2d) Trainium2 kernel.

Math (reference): x(B=64,IC=16,32,32) -> y=x.reshape(IC,B,N=1024)[:,:,bitrev];
broadcast over OC=32; 10 radix-2 butterfly layers with per-(ic,oc) twiddles;
mean over ic; + bias -> (B,OC,32,32).

Strategy:
  * Shard over OC: 8 cores x 4 oc each; every core holds all 16 ic so the
    ic-mean is core-local (no collective). Host concatenates oc slices.
  * Weight transform (host, layout/compose only -- analogous to a Winograd
    filter transform): butterfly layers 0..7 compose into dense 256x256
    block-diagonal matrices (4 blocks per (ic,oc)). TensorE applies them as
    K=128 matmuls with PSUM K-accumulation (16 MMs per (ic,oc), N=64).
  * Layers 8,9 pair elements across 512/256 strides = across free-dim chunks
    in the device layout [128 partitions = n%128, free = (chunk n//128, b)].
    VectorE does them with scalar_tensor_tensor using per-partition twiddle
    vectors ([P,1] APs, no broadcast/replication needed), then accumulates
    the ic-mean (1/16 folded into layer-9 coeffs) and bias.

Device layout: y[ic] tile [128, 512]: partition p = n & 127, free = c*64 + b
with chunk c = n >> 7 (3 bits: n9 n8 n7).
"""

import numpy as np

B, IC, OC, H, W = 64, 16, 32, 32, 32
N = H * W          # 1024
M = 10             # butterfly layers
NCORES = 8
OCL = OC // NCORES  # 4 oc per core
NCH = 8            # free-dim chunks (n9n8n7)
P = 128            # partitions (n6..n0)
SB = 256           # composed stage-A block size (layers 0..7)
NBLK = N // SB     # 4 blocks per (ic,oc)

# stage-A (matmul) dtype: np.float32 or ml_dtypes.bfloat16 (set in W_DTYPE)
W_DTYPE = np.float32


def _bitrev(n):
    bits = int(np.log2(n))
    idx = np.arange(n, dtype=np.int64)
    rev = np.zeros(n, dtype=np.int64)
    for b in range(bits):
        rev = (rev << 1) | ((idx >> b) & 1)
    return rev


def _compose_stageA(tw):
    """Compose butterfly layers 0..7 into A[ic,oc,g,256,256] (g=4 blocks).

    Layer l (stride s=2^l) acts on flat index k*2s + q*s + j; for l<=7 the
    mixing stays inside 256-aligned blocks.
    """
    ic, oc = tw.shape[0], tw.shape[1]
    A = np.zeros((ic, oc, NBLK, SB, SB), dtype=np.float32)
    eye = np.eye(SB, dtype=np.float32)
    A[:] = eye  # broadcast
    for l in range(8):
        s = 1 << l
        nb_loc = SB // (2 * s)  # local pair-block count inside a 256 block
        # twiddle layer l: (ic, oc, nb, s, 2, 2) with nb = N//(2s)
        t = tw[:, :, l].reshape(ic, oc, N // (2 * s), s, 2, 2)
        # local slice for block g: global k = g*nb_loc + k_loc
        t = t.reshape(ic, oc, NBLK, nb_loc, s, 2, 2)
        # A view: (ic, oc, g, k_loc, 2, s, SB) rows evolve
        Av = A.reshape(ic, oc, NBLK, nb_loc, 2, s, SB)
        a0 = Av[:, :, :, :, 0]  # (ic,oc,g,k,s,SB)
        a1 = Av[:, :, :, :, 1]
        t00 = t[..., 0, 0, None]  # (ic,oc,g,k,s,1)
        t01 = t[..., 0, 1, None]
        t10 = t[..., 1, 0, None]
        t11 = t[..., 1, 1, None]
        new0 = t00 * a0 + t01 * a1
        new1 = t10 * a0 + t11 * a1
        Av[:, :, :, :, 0] = new0
        Av[:, :, :, :, 1] = new1
    return A


def _stageB_coeffs(tw):
    """Per-partition coefficient vectors for layers 8 and 9.

    Returns tb[ic, oc, 2(layer), 8(out chunk c'), 2(q), 128(p)] float32,
    with the 1/IC mean folded into layer 9, and the input-chunk index map.
    layer 8 (s=256): flat = k*512 + q*256 + j, j=(n7,p);  out chunk
      c' = (k, p_out, n7) -> reads chunks (k,q,n7), coeff t8[k, (n7,p), p_out, q].
    layer 9 (s=512): flat = q*512 + j, j=(n8,n7,p); out c' = (p_out, n8, n7)
      -> reads chunks (q, n8, n7), coeff t9[0, (n8,n7,p), p_out, q]/16.
    """
    ic, oc = tw.shape[0], tw.shape[1]
    t8 = tw[:, :, 8].reshape(ic, oc, 2, 256, 2, 2)   # [k, j, p_out, q]
    t9 = tw[:, :, 9].reshape(ic, oc, 1, 512, 2, 2)
    tb = np.zeros((ic, oc, 2, NCH, 2, P), dtype=np.float32)
    src = np.zeros((2, NCH, 2), dtype=np.int64)
    pr = np.arange(P)
    for cp in range(NCH):
        k, p_out, n7 = cp >> 2, (cp >> 1) & 1, cp & 1
        for q in range(2):
            tb[:, :, 0, cp, q] = t8[:, :, k, n7 * 128 + pr, p_out, q]
            src[0, cp, q] = k * 4 + q * 2 + n7
        p_out9, n8, n7_ = cp >> 2, (cp >> 1) & 1, cp & 1
        for q in range(2):
            tb[:, :, 1, cp, q] = t9[:, :, 0, (cp & 3) * 128 + pr, p_out9, q] / IC
            src[1, cp, q] = q * 4 + (cp & 3)
    return tb, src


_SRC8 = None  # filled lazily (static chunk-index map, twiddle-independent)


def _prep_host(x, twiddle, bias):
    """All host-side layout work. Returns per-core input maps (numpy)."""
    perm = _bitrev(N)
    y = np.ascontiguousarray(x).reshape(IC, B, N)[:, :, perm]
    # device layout y[ic, p, c*64+b]
    y_dev = np.ascontiguousarray(
        y.reshape(IC, B, NCH, P).transpose(0, 3, 2, 1)
    ).reshape(IC, P, NCH * B)

    A = _compose_stageA(np.asarray(twiddle, dtype=np.float32))
    tb, src = _stageB_coeffs(np.asarray(twiddle, dtype=np.float32))

    bias_np = np.asarray(bias, dtype=np.float32).reshape(OC, NCH, P)

    in_maps = []
    for core in range(NCORES):
        osl = slice(core * OCL, (core + 1) * OCL)
        Ac = A[:, osl]  # (IC, OCL, 4, 256, 256)
        # lhsT tiles: w[ic,o,p_k, g, h, kin, m] = Ac[ic,o,g][h*128+m, kin*128+p_k]
        w = np.ascontiguousarray(
            Ac.reshape(IC, OCL, NBLK, 2, P, 2, P)  # [g, h, m, kin, k]
            .transpose(0, 1, 6, 2, 3, 5, 4)        # [ic,o,k,g,h,kin,m]
        ).astype(W_DTYPE)
        tbc = np.ascontiguousarray(
            tb[:, osl].transpose(0, 1, 5, 2, 3, 4)  # [ic,o,p,l,c',q]
        ).reshape(IC, OCL, P, 32).astype(np.float32)
        bc = np.ascontiguousarray(
            np.broadcast_to(
                bias_np[osl].transpose(0, 2, 1)[:, :, :, None], (OCL, P, NCH, B)
            )
        ).reshape(OCL, P, NCH * B).astype(np.float32)
        in_maps.append(
            {
                "y": y_dev.astype(W_DTYPE),
                "w": w.reshape(IC, OCL, P, NBLK * 2 * 2 * P),
                "tb": tbc,
                "bias": bc,
            }
        )
    return in_maps, src


def _emulate_core(im, src):
    """Numpy emulation of the device program (for validating layout math)."""
    y = im["y"].astype(np.float32)      # (IC, 128, 512)
    w = im["w"].astype(np.float32).reshape(IC, OCL, P, NBLK, 2, 2, P)
    tb = im["tb"].reshape(IC, OCL, P, 2, NCH, 2)
    out = np.array(im["bias"], dtype=np.float32).reshape(OCL, P, NCH, B).copy()
    for o in range(OCL):
        for ic in range(IC):
            z = np.zeros((P, NCH, B), dtype=np.float32)
            yv = y[ic].reshape(P, NCH, B)
            for g in range(NBLK):
                for h in range(2):
                    acc = np.zeros((P, B), dtype=np.float32)
                    for kin in range(2):
                        lhsT = w[ic, o, :, g, h, kin]  # [k, m]
                        acc += lhsT.T @ yv[:, 2 * g + kin]
                    z[:, 2 * g + h] = acc
            y8 = np.zeros_like(z)
            for cp in range(NCH):
                y8[:, cp] = (
                    tb[ic, o, :, 0, cp, 0, None] * z[:, src[0, cp, 0]]
                    + tb[ic, o, :, 0, cp, 1, None] * z[:, src[0, cp, 1]]
                )
            for cp in range(NCH):
                out[o, :, cp] += (
                    tb[ic, o, :, 1, cp, 0, None] * y8[:, src[1, cp, 0]]
                    + tb[ic, o, :, 1, cp, 1, None] * y8[:, src[1, cp, 1]]
                )
    return out.reshape(OCL, P, NCH * B)


def _build_program(src):
    import concourse.bacc as bacc
    import concourse.mybir as mybir
    from concourse.tile import TileContext

    wdt = mybir.dt.bfloat16 if W_DTYPE != np.float32 else mybir.dt.float32
    f32 = mybir.dt.float32
    MULT, ADD = mybir.AluOpType.mult, mybir.AluOpType.add

    nc = bacc.Bacc(None, target_bir_lowering=False)
    y_d = nc.dram_tensor("y", (IC, P, NCH * B), wdt, kind="ExternalInput")
    w_d = nc.dram_tensor("w", (IC, OCL, P, NBLK * 4 * P), wdt, kind="ExternalInput")
    tb_d = nc.dram_tensor("tb", (IC, OCL, P, 32), f32, kind="ExternalInput")
    bias_d = nc.dram_tensor("bias", (OCL, P, NCH * B), f32, kind="ExternalInput")
    o_d = nc.dram_tensor("o", (OCL, P, NCH * B), f32, kind="ExternalOutput")

    with TileContext(nc) as tc:
        with (
            tc.tile_pool(name="ypool", bufs=2) as ypool,
            tc.tile_pool(name="wpool", bufs=3) as wpool,
            tc.tile_pool(name="tbpool", bufs=3) as tbpool,
            tc.tile_pool(name="accpool", bufs=OCL) as accpool,
            tc.tile_pool(name="y8pool", bufs=3) as y8pool,
            tc.tile_pool(name="psum", bufs=4, space="PSUM") as pspool,
        ):
            accs = []
            for o in range(OCL):
                acc = accpool.tile([P, NCH * B], f32, tag="acc")
                nc.sync.dma_start(out=acc[:], in_=bias_d[o])
                accs.append(acc)
            for ic in range(IC):
                ytile = ypool.tile([P, NCH * B], wdt)
                nc.sync.dma_start(out=ytile[:], in_=y_d[ic])
                for o in range(OCL):
                    wtile = wpool.tile([P, NBLK * 4 * P], wdt)
                    nc.sync.dma_start(out=wtile[:], in_=w_d[ic, o])
                    tbt = tbpool.tile([P, 32], f32)
                    nc.sync.dma_start(out=tbt[:], in_=tb_d[ic, o])
                    z = pspool.tile([P, NCH * B], f32)
                    for g in range(NBLK):
                        for h in range(2):
                            cp = 2 * g + h
                            for kin in range(2):
                                wi = ((g * 2 + h) * 2 + kin) * P
                                nc.tensor.matmul(
                                    z[:, cp * B : (cp + 1) * B],
                                    wtile[:, wi : wi + P],
                                    ytile[:, (2 * g + kin) * B : (2 * g + kin + 1) * B],
                                    start=(kin == 0),
                                    stop=(kin == 1),
                                )
                    y8 = y8pool.tile([P, NCH * B], f32)
                    for cp in range(NCH):
                        s0, s1 = int(src[0, cp, 0]), int(src[0, cp, 1])
                        osl = slice(cp * B, (cp + 1) * B)
                        # y8_cp = t_q1 * z_s1 ; then y8_cp = (z_s0*t_q0) + y8_cp
                        nc.vector.tensor_scalar_mul(
                            y8[:, osl],
                            z[:, s1 * B : (s1 + 1) * B],
                            tbt[:, (0 * NCH + cp) * 2 + 1 : (0 * NCH + cp) * 2 + 2],
                        )
                        nc.vector.scalar_tensor_tensor(
                            y8[:, osl],
                            z[:, s0 * B : (s0 + 1) * B],
                            tbt[:, (0 * NCH + cp) * 2 : (0 * NCH + cp) * 2 + 1],
                            y8[:, osl],
                            MULT,
                            ADD,
                        )
                    # Layer 9 runs on the otherwise-idle ScalarE (per-partition
                    # scale via activation) + GpSimdE (fused mul-add and the
                    # ic-accumulation), leaving DVE only layer 8 (PSUM reads).
                    yo = y8pool.tile([P, NCH * B], f32, tag="yo")
                    for cp in range(NCH):
                        s0, s1 = int(src[1, cp, 0]), int(src[1, cp, 1])
                        osl = slice(cp * B, (cp + 1) * B)
                        nc.scalar.activation(
                            yo[:, osl],
                            y8[:, s1 * B : (s1 + 1) * B],
                            mybir.ActivationFunctionType.Copy,
                            scale=tbt[:, (1 * NCH + cp) * 2 + 1 : (1 * NCH + cp) * 2 + 2],
                        )
                        nc.vector.scalar_tensor_tensor(
                            yo[:, osl],
                            y8[:, s0 * B : (s0 + 1) * B],
                            tbt[:, (1 * NCH + cp) * 2 : (1 * NCH + cp) * 2 + 1],
                            yo[:, osl],
                            MULT,
                            ADD,
                        )
                    nc.vector.tensor_add(accs[o][:], accs[o][:], yo[:])
            for o in range(OCL):
                nc.sync.dma_start(out=o_d[o], in_=accs[o][:])
    nc.finalize()
    return nc


_LAST_RESULTS = {"exec_time_ns": None}


def kernel(x, twiddle, bias, _trace=False, _emulate=False):
    in_maps, src = _prep_host(
        np.asarray(x), np.asarray(twiddle), np.asarray(bias)
    )
    if _emulate:
        outs = [_emulate_core(im, src) for im in in_maps]
    else:
        from concourse.bass_utils import run_bass_kernel_spmd

        nc = _build_program(src)
        res = run_bass_kernel_spmd(
            nc, in_maps, list(range(NCORES)), trace=_trace
        )
        _LAST_RESULTS["exec_time_ns"] = res.exec_time_ns
        _LAST_RESULTS["mean_exec_time_ns"] = res.mean_exec_time_ns
        outs = [r["o"] for r in res.results]
    # o[oc_l, p, c*64+b] -> (OC, B, N); final (B,OC,H,W) is a pure
    # reinterpret of (OC,B,N) bytes (reference uses .reshape, not transpose).
    full = np.concatenate(
        [
            np.asarray(o, dtype=np.float32)
            .reshape(OCL, P, NCH, B)
            .transpose(0, 3, 2, 1)
            .reshape(OCL, B, N)
            for o in outs
        ],
        axis=0,
    )
    return np.ascontiguousarray(full).reshape(B, OC, H, W).astype(np.float32)



# revision 6
# speedup vs baseline: 2.0122x; 2.0122x over previous
"""Butterfly-Conv2d (nn_BConv2d) Trainium2 kernel — v2 (low-precision, PE-accumulated).

Math (reference): x(B=64,IC=16,32,32) -> y=x.reshape(IC,B,N=1024)[:,:,bitrev];
broadcast over OC=32; 10 radix-2 butterfly layers with per-(ic,oc) twiddles;
mean over ic; + bias -> (B,OC,32,32).

Strategy (per core: all 16 ic x 4 oc, core-local ic-mean, no collective):
  * Host (free): compose butterfly layers 0..7 into dense 256x256 blocks
    (4 per (ic,oc)); cast weights to fp8/bf16. Build per-(p,chunk) coeff
    vectors for layers 8/9 (1/16 mean folded into layer 9).
  * Stage A (PE): 16 matmuls per (ic,oc) [k=128, m=128, free=64] with PSUM
    K-accumulation -> z[p, (cp,b)] f32 in PSUM.
  * Act: transpose-convert z -> SBUF bf16 in (b, n9, n8, n7) free layout
    (coefficient broadcasts then have packed last dims -> DVE 4x mode).
  * Stage B (DVE): 5 wide scalar_tensor_tensor ops per (ic,oc):
      u_q = z[b, n9, q, n7] * c8_q[n9, n8', n7]   (q=0,1; broadcast over b)
      y8  = u0 + u1
      v_q = y8[b, q, n8', n7] * c9_q[n9', n8', n7]
    (stt with immediate scalar 1.0 => InstTensorScalarPtr, 4x perf mode)
  * ic-mean accumulation (PE): acc_o += I @ v_q, 32 accumulating matmuls
    into a persistent PSUM bank per oc.
  * Epilogue: out_o = acc_o + bias (DVE), DMA out.

Device output layout: o[ocl, p, b*8+cp] with n = cp*128+p.
"""

import numpy as np
import ml_dtypes

B, IC, OC, H, W = 64, 16, 32, 32, 32
N = H * W          # 1024
M = 10             # butterfly layers
NCORES = 8
OCL = OC // NCORES  # 4 oc per core
NCH = 8            # free-dim chunks (n9n8n7)
P = 128            # partitions (n6..n0)
SB = 256           # composed stage-A block size (layers 0..7)
NBLK = N // SB     # 4 blocks per (ic,oc)

# stage-A weight dtype: "fp8" (float8_e4m3) or "bf16"
W_DT = "bf16"
Y_DT = "bf16"      # stage-A rhs dtype (must be 8-bit if PE requires match)

_NPDT = {"fp8": ml_dtypes.float8_e4m3, "bf16": ml_dtypes.bfloat16}


def _bitrev(n):
    bits = int(np.log2(n))
    idx = np.arange(n, dtype=np.int64)
    rev = np.zeros(n, dtype=np.int64)
    for b in range(bits):
        rev = (rev << 1) | ((idx >> b) & 1)
    return rev


def _compose_stageA(tw):
    """Compose butterfly layers 0..7 into A[ic,oc,g,256,256] (g=4 blocks)."""
    ic, oc = tw.shape[0], tw.shape[1]
    A = np.zeros((ic, oc, NBLK, SB, SB), dtype=np.float32)
    eye = np.eye(SB, dtype=np.float32)
    A[:] = eye
    for l in range(8):
        s = 1 << l
        nb_loc = SB // (2 * s)
        t = tw[:, :, l].reshape(ic, oc, N // (2 * s), s, 2, 2)
        t = t.reshape(ic, oc, NBLK, nb_loc, s, 2, 2)
        Av = A.reshape(ic, oc, NBLK, nb_loc, 2, s, SB)
        a0 = Av[:, :, :, :, 0]
        a1 = Av[:, :, :, :, 1]
        t00 = t[..., 0, 0, None]
        t01 = t[..., 0, 1, None]
        t10 = t[..., 1, 0, None]
        t11 = t[..., 1, 1, None]
        new0 = t00 * a0 + t01 * a1
        new1 = t10 * a0 + t11 * a1
        Av[:, :, :, :, 0] = new0
        Av[:, :, :, :, 1] = new1
    return A


def _stageB_coeffs(tw):
    """cf[ic, oc, p, 32] f32: 4 groups of 8 chunk-coeffs per partition.

    group j=0/1: layer-8 q=0/1 coeff, stored n8'-major (n8', n9, n7) so the
        per-n8' slice is contiguous (HW AP limit: 2 free dims/operand):
        cf[.., q, n8o*4+n9*2+n7] = t8[k=n9, n7*128+p, n8', q]
    group j=2/3: layer-9 q9=0/1 coeff, stored n9'-major (n9', n8', n7):
        cf[.., 2+q, n9o*4+n8o*2+n7] = t9[0, n8'*256+n7*128+p, n9', q9] / IC
    """
    ic, oc = tw.shape[0], tw.shape[1]
    t8 = tw[:, :, 8].reshape(ic, oc, 2, 256, 2, 2)   # [k, j, p_out, q]
    t9 = tw[:, :, 9].reshape(ic, oc, 512, 2, 2)      # [j, p_out, q]
    cf = np.zeros((ic, oc, P, 4, 8), dtype=np.float32)
    pr = np.arange(P)
    for n9 in range(2):
        for n8o in range(2):
            for n7 in range(2):
                for q in range(2):
                    cf[:, :, :, q, n8o * 4 + n9 * 2 + n7] = t8[
                        :, :, n9, n7 * 128 + pr, n8o, q
                    ]
                    cf[:, :, :, 2 + q, n9 * 4 + n8o * 2 + n7] = (
                        t9[:, :, n8o * 256 + n7 * 128 + pr, n9, q] / IC
                    )
    return cf.reshape(ic, oc, P, 32)


def _prep_host(x, twiddle, bias):
    """All host-side layout work. Returns per-core input maps (numpy)."""
    wnp = _NPDT[W_DT]
    ynp = _NPDT[Y_DT]
    perm = _bitrev(N)
    y = np.ascontiguousarray(x).reshape(IC, B, N)[:, :, perm]
    # device layout y[ic, p, c*64+b]
    y_dev = np.ascontiguousarray(
        y.reshape(IC, B, NCH, P).transpose(0, 3, 2, 1)
    ).reshape(IC, P, NCH * B).astype(ynp)

    A = _compose_stageA(np.asarray(twiddle, dtype=np.float32))
    cf = _stageB_coeffs(np.asarray(twiddle, dtype=np.float32))

    # bias in device (b,cp) layout: bias_dev[oc, p, b*8+cp] = bias[oc, cp*128+p]
    bias_pc = np.asarray(bias, dtype=np.float32).reshape(OC, NCH, P)
    bias_dev = np.broadcast_to(
        bias_pc.transpose(0, 2, 1)[:, :, None, :], (OC, P, B, NCH)
    ).reshape(OC, P, NCH * B)

    ident = np.eye(P, dtype=np.float32).astype(ml_dtypes.bfloat16)

    in_maps = []
    for core in range(NCORES):
        osl = slice(core * OCL, (core + 1) * OCL)
        Ac = A[:, osl]  # (IC, OCL, 4, 256, 256)
        # lhsT tiles: w[ic,o,p_k, g, h, kin, m] = Ac[ic,o,g][h*128+m, kin*128+p_k]
        w = np.ascontiguousarray(
            Ac.reshape(IC, OCL, NBLK, 2, P, 2, P)  # [g, h, m, kin, k]
            .transpose(0, 1, 6, 2, 3, 5, 4)        # [ic,o,k,g,h,kin,m]
        ).astype(wnp)
        in_maps.append(
            {
                "y": y_dev,
                "w": w.reshape(IC, OCL, P, NBLK * 4 * P),
                "cf": np.ascontiguousarray(cf[:, osl]).astype(ml_dtypes.bfloat16),
                "bias": np.ascontiguousarray(bias_dev[osl]),
                "ident": ident,
            }
        )
    return in_maps


def _emulate_core(im):
    """Numpy emulation of the device program (for validating layout math)."""
    y = im["y"].astype(np.float32)      # (IC, 128, 512) free=(cp,b)
    w = im["w"].astype(np.float32).reshape(IC, OCL, P, NBLK, 2, 2, P)
    cf = im["cf"].astype(np.float32).reshape(IC, OCL, P, 4, 8)
    out = np.array(im["bias"], dtype=np.float32).reshape(OCL, P, B, NCH).copy()
    bf = lambda a: a.astype(ml_dtypes.bfloat16).astype(np.float32)
    for o in range(OCL):
        acc = np.zeros((P, B, NCH), dtype=np.float32)
        for ic in range(IC):
            yv = y[ic].reshape(P, NCH, B)
            z = np.zeros((P, NCH, B), dtype=np.float32)
            for g in range(NBLK):
                for h in range(2):
                    a = np.zeros((P, B), dtype=np.float32)
                    for kin in range(2):
                        lhsT = w[ic, o, :, g, h, kin]  # [k, m]
                        a += lhsT.T @ yv[:, 2 * g + kin]
                    z[:, 2 * g + h] = a
            # Act transpose-convert -> zb[p, b, n9, n8, n7] bf16
            zb = bf(z.reshape(P, 2, 2, 2, B).transpose(0, 4, 1, 2, 3))
            c = cf[ic, o].reshape(P, 1, 4, 2, 2, 2)
            c8_0 = c[:, :, 0].transpose(0, 1, 3, 2, 4)  # (n8',n9,n7)->(n9,n8',n7)
            c8_1 = c[:, :, 1].transpose(0, 1, 3, 2, 4)
            u0 = bf(zb[:, :, :, 0:1, :] * c8_0)
            u1 = bf(zb[:, :, :, 1:2, :] * c8_1)
            y8 = bf(u0 + u1)                          # [p, b, n9, n8', n7]
            v0 = bf(y8[:, :, 0:1] * c[:, :, 2])
            v1 = bf(y8[:, :, 1:2] * c[:, :, 3])
            acc += (v0 + v1).reshape(P, B, NCH)
        out[o] += acc
    return out.reshape(OCL, P, NCH * B)


def _build_program():
    import concourse.bacc as bacc
    import concourse.mybir as mybir
    from concourse.tile import TileContext

    f32 = mybir.dt.float32
    bf16 = mybir.dt.bfloat16
    wdt = mybir.dt.float8e4 if W_DT == "fp8" else mybir.dt.bfloat16
    ydt = mybir.dt.float8e4 if Y_DT == "fp8" else mybir.dt.bfloat16
    MULT, ADD = mybir.AluOpType.mult, mybir.AluOpType.add
    COPY = mybir.ActivationFunctionType.Copy

    nc = bacc.Bacc(None, target_bir_lowering=False)
    y_d = nc.dram_tensor("y", (IC, P, NCH * B), ydt, kind="ExternalInput")
    w_d = nc.dram_tensor("w", (IC, OCL, P, NBLK * 4 * P), wdt, kind="ExternalInput")
    cf_d = nc.dram_tensor("cf", (IC, OCL, P, 32), bf16, kind="ExternalInput")
    bias_d = nc.dram_tensor("bias", (OCL, P, NCH * B), f32, kind="ExternalInput")
    id_d = nc.dram_tensor("ident", (P, P), bf16, kind="ExternalInput")
    o_d = nc.dram_tensor("o", (OCL, P, NCH * B), f32, kind="ExternalOutput")

    with TileContext(nc) as tc:
        with (
            tc.tile_pool(name="ypool", bufs=2) as ypool,
            tc.tile_pool(name="wpool", bufs=3) as wpool,
            tc.tile_pool(name="cfpool", bufs=3) as cfpool,
            tc.tile_pool(name="zbpool", bufs=2) as zbpool,
            tc.tile_pool(name="upool", bufs=2) as upool,
            tc.tile_pool(name="vpool", bufs=2) as vpool,
            tc.tile_pool(name="misc", bufs=1) as misc,
            tc.tile_pool(name="zpsum", bufs=3, space="PSUM") as zpsum,
            tc.tile_pool(name="apsum", bufs=OCL, space="PSUM") as apsum,
        ):
            ident = misc.tile([P, P], bf16, tag="ident")
            nc.sync.dma_start(out=ident[:], in_=id_d[:, :])
            accs = []
            for o in range(OCL):
                acc = apsum.tile([P, NCH * B], f32, tag="acc")
                accs.append(acc)

            pend = None  # deferred acc-matmuls: (o, v0, v1, first)

            def flush_pend():
                nonlocal pend
                if pend is None:
                    return
                o, v0, v1, first, last = pend
                nc.tensor.matmul(
                    accs[o][:],
                    ident[:],
                    v0[:].rearrange("p b x y z -> p (b x y z)"),
                    start=first, stop=False, skip_group_check=True,
                )
                nc.tensor.matmul(
                    accs[o][:],
                    ident[:],
                    v1[:].rearrange("p b x y z -> p (b x y z)"),
                    start=False, stop=last, skip_group_check=True,
                )
                pend = None

            for ic in range(IC):
                ytile = ypool.tile([P, NCH * B], ydt)
                nc.sync.dma_start(out=ytile[:], in_=y_d[ic])
                for o in range(OCL):
                    wtile = wpool.tile([P, NBLK * 4 * P], wdt)
                    nc.sync.dma_start(out=wtile[:], in_=w_d[ic, o])
                    cft = cfpool.tile([P, 4, 8], bf16)
                    nc.sync.dma_start(
                        out=cft[:], in_=cf_d[ic, o].rearrange("p (j c) -> p j c", j=4)
                    )
                    z = zpsum.tile([P, NCH * B], f32)
                    for g in range(NBLK):
                        for h in range(2):
                            cp = 2 * g + h
                            for kin in range(2):
                                wi = ((g * 2 + h) * 2 + kin) * P
                                nc.tensor.matmul(
                                    z[:, cp * B : (cp + 1) * B],
                                    wtile[:, wi : wi + P],
                                    ytile[:, (2 * g + kin) * B : (2 * g + kin + 1) * B],
                                    start=(kin == 0),
                                    stop=(kin == 1),
                                )
                    # deferred acc-MMs of the PREVIOUS pair go after this
                    # pair's z-MMs so PE never waits on DVE head-of-line.
                    flush_pend()

                    # Act: transpose-convert z (cp,b) f32 -> zb (b,n9,n8,n7) bf16
                    zb = zbpool.tile([P, B, 2, 2, 2], bf16)
                    zb_cpb = zb[:].transpose([0, 2, 3, 4, 1])
                    z_v = z[:].rearrange("p (x y w b) -> p x y w b", x=2, y=2, w=2)
                    nc.scalar.activation(zb_cpb, z_v, COPY, scale=1.0)

                    # DVE stage B: wide stt ops (4x perf mode). The HW AP limit
                    # is 3 free dims, so each product op is split over the bit
                    # that is broadcast in its gather operand (2 ops of 256).
                    cg = cft[:].rearrange("p j (x y z) -> p j x y z", x=2, y=2)
                    u0 = upool.tile([P, B, 2, 2, 2], bf16, tag="u0")
                    u1 = upool.tile([P, B, 2, 2, 2], bf16, tag="u1")
                    for q, ut in ((0, u0), (1, u1)):
                        # u_q[p,b,n9,n8o,n7] = zb[p,b,n9,q,n7] * c8q[p,n8o,n9,n7]
                        zg = zb[:, :, :, q, :]                  # [P,B,2,2]
                        for n8o in range(2):
                            cq = cg[:, q, n8o].unsqueeze(1).broadcast_to(
                                (P, B, 2, 2)
                            )
                            nc.vector.scalar_tensor_tensor(
                                ut[:, :, :, n8o, :], zg, 1.0, cq, MULT, MULT
                            )
                    y8 = upool.tile([P, B, 2, 2, 2], bf16, tag="y8")
                    nc.vector.scalar_tensor_tensor(
                        y8[:].rearrange("p b x y z -> p (b x y z)"),
                        u0[:].rearrange("p b x y z -> p (b x y z)"),
                        1.0,
                        u1[:].rearrange("p b x y z -> p (b x y z)"),
                        MULT, ADD,
                    )
                    v0 = vpool.tile([P, B, 2, 2, 2], bf16, tag="v0")
                    v1 = vpool.tile([P, B, 2, 2, 2], bf16, tag="v1")
                    for q, vt in ((0, v0), (1, v1)):
                        # v_q[p,b,n9o,n8o,n7] = y8[p,b,q,n8o,n7] * c9q[p,n9o,n8o,n7]
                        yg = y8[:, :, q, :, :]                  # [P,B,2,2]
                        for n9o in range(2):
                            cq = cg[:, 2 + q, n9o].unsqueeze(1).broadcast_to(
                                (P, B, 2, 2)
                            )
                            nc.vector.scalar_tensor_tensor(
                                vt[:, :, n9o, :, :], yg, 1.0, cq, MULT, MULT
                            )
                    pend = (o, v0, v1, ic == 0, ic == IC - 1)
            flush_pend()

            for o in range(OCL):
                biast = misc.tile([P, NCH * B], f32, tag=f"bias{o}")
                nc.sync.dma_start(out=biast[:], in_=bias_d[o])
                outt = misc.tile([P, NCH * B], f32, tag=f"out{o}")
                nc.vector.scalar_tensor_tensor(
                    outt[:], accs[o][:], 1.0, biast[:], MULT, ADD
                )
                nc.sync.dma_start(out=o_d[o], in_=outt[:])
    nc.finalize()
    return nc


_LAST_RESULTS = {"exec_time_ns": None}


def kernel(x, twiddle, bias, _trace=False, _emulate=False):
    in_maps = _prep_host(np.asarray(x), np.asarray(twiddle), np.asarray(bias))
    if _emulate:
        outs = [_emulate_core(im) for im in in_maps]
    else:
        from concourse.bass_utils import run_bass_kernel_spmd

        nc = _build_program()
        res = run_bass_kernel_spmd(nc, in_maps, list(range(NCORES)), trace=_trace)
        _LAST_RESULTS["exec_time_ns"] = res.exec_time_ns
        _LAST_RESULTS["mean_exec_time_ns"] = res.mean_exec_time_ns
        outs = [r["o"] for r in res.results]
    # o[oc_l, p, b*8+cp] -> (OC, B, N) with n = cp*128+p; final (B,OC,H,W)
    # is a pure reinterpret of (OC,B,N) bytes (reference uses .reshape).
    full = np.concatenate(
        [
            np.asarray(o, dtype=np.float32)
            .reshape(OCL, P, B, NCH)
            .transpose(0, 2, 3, 1)
            .reshape(OCL, B, N)
            for o in outs
        ],
        axis=0,
    )
    return np.ascontiguousarray(full).reshape(B, OC, H, W).astype(np.float32)


# revision 10
# speedup vs baseline: 2.5530x; 1.2688x over previous
"""Butterfly-Conv2d (nn_BConv2d) Trainium2 kernel — v2 (low-precision, PE-accumulated).

Math (reference): x(B=64,IC=16,32,32) -> y=x.reshape(IC,B,N=1024)[:,:,bitrev];
broadcast over OC=32; 10 radix-2 butterfly layers with per-(ic,oc) twiddles;
mean over ic; + bias -> (B,OC,32,32).

Strategy (per core: all 16 ic x 4 oc, core-local ic-mean, no collective):
  * Host (free): compose butterfly layers 0..7 into dense 256x256 blocks
    (4 per (ic,oc)); cast weights to fp8/bf16. Build per-(p,chunk) coeff
    vectors for layers 8/9 (1/16 mean folded into layer 9).
  * Stage A (PE): 16 matmuls per (ic,oc) [k=128, m=128, free=64] with PSUM
    K-accumulation -> z[p, (cp,b)] f32 in PSUM.
  * Act: transpose-convert z -> SBUF bf16 in (b, n9, n8, n7) free layout
    (coefficient broadcasts then have packed last dims -> DVE 4x mode).
  * Stage B (DVE): 5 wide scalar_tensor_tensor ops per (ic,oc):
      u_q = z[b, n9, q, n7] * c8_q[n9, n8', n7]   (q=0,1; broadcast over b)
      y8  = u0 + u1
      v_q = y8[b, q, n8', n7] * c9_q[n9', n8', n7]
    (stt with immediate scalar 1.0 => InstTensorScalarPtr, 4x perf mode)
  * ic-mean accumulation (PE): acc_o += I @ v_q, 32 accumulating matmuls
    into a persistent PSUM bank per oc.
  * Epilogue: out_o = acc_o + bias (DVE), DMA out.

Device output layout: o[ocl, p, b*8+cp] with n = cp*128+p.
"""

import numpy as np
import ml_dtypes

B, IC, OC, H, W = 64, 16, 32, 32, 32
N = H * W          # 1024
M = 10             # butterfly layers
NCORES = 8
OCL = OC // NCORES  # 4 oc per core
NCH = 8            # free-dim chunks (n9n8n7)
P = 128            # partitions (n6..n0)
SB = 256           # composed stage-A block size (layers 0..7)
NBLK = N // SB     # 4 blocks per (ic,oc)

# stage-A weight dtype: "fp8" (float8_e4m3) or "bf16"
W_DT = "bf16"
Y_DT = "bf16"      # stage-A rhs dtype (mixed fp8 lhsT x bf16 rhs verified on HW)

_NPDT = {"fp8": ml_dtypes.float8_e4m3, "bf16": ml_dtypes.bfloat16}


def _bitrev(n):
    bits = int(np.log2(n))
    idx = np.arange(n, dtype=np.int64)
    rev = np.zeros(n, dtype=np.int64)
    for b in range(bits):
        rev = (rev << 1) | ((idx >> b) & 1)
    return rev


def _compose_stageA(tw):
    """Compose butterfly layers 0..7 into A[ic,oc,g,256,256] (g=4 blocks)."""
    ic, oc = tw.shape[0], tw.shape[1]
    A = np.zeros((ic, oc, NBLK, SB, SB), dtype=np.float32)
    eye = np.eye(SB, dtype=np.float32)
    A[:] = eye
    for l in range(8):
        s = 1 << l
        nb_loc = SB // (2 * s)
        t = tw[:, :, l].reshape(ic, oc, N // (2 * s), s, 2, 2)
        t = t.reshape(ic, oc, NBLK, nb_loc, s, 2, 2)
        Av = A.reshape(ic, oc, NBLK, nb_loc, 2, s, SB)
        a0 = Av[:, :, :, :, 0]
        a1 = Av[:, :, :, :, 1]
        t00 = t[..., 0, 0, None]
        t01 = t[..., 0, 1, None]
        t10 = t[..., 1, 0, None]
        t11 = t[..., 1, 1, None]
        new0 = t00 * a0 + t01 * a1
        new1 = t10 * a0 + t11 * a1
        Av[:, :, :, :, 0] = new0
        Av[:, :, :, :, 1] = new1
    return A


def _stageB_coeffs(tw):
    """cf[ic, oc, p, 32] f32: 4 groups of 8 chunk-coeffs per partition.

    group j=0/1: layer-8 q=0/1 coeff, stored n8'-major (n8', n9, n7) so the
        per-n8' slice is contiguous (HW AP limit: 2 free dims/operand):
        cf[.., q, n8o*4+n9*2+n7] = t8[k=n9, n7*128+p, n8', q]
    group j=2/3: layer-9 q9=0/1 coeff, stored n9'-major (n9', n8', n7):
        cf[.., 2+q, n9o*4+n8o*2+n7] = t9[0, n8'*256+n7*128+p, n9', q9] / IC
    """
    ic, oc = tw.shape[0], tw.shape[1]
    t8 = tw[:, :, 8].reshape(ic, oc, 2, 256, 2, 2)   # [k, j, p_out, q]
    t9 = tw[:, :, 9].reshape(ic, oc, 512, 2, 2)      # [j, p_out, q]
    cf = np.zeros((ic, oc, P, 4, 8), dtype=np.float32)
    pr = np.arange(P)
    for n9 in range(2):
        for n8o in range(2):
            for n7 in range(2):
                for q in range(2):
                    cf[:, :, :, q, n8o * 4 + n9 * 2 + n7] = t8[
                        :, :, n9, n7 * 128 + pr, n8o, q
                    ]
                    cf[:, :, :, 2 + q, n9 * 4 + n8o * 2 + n7] = (
                        t9[:, :, n8o * 256 + n7 * 128 + pr, n9, q] / IC
                    )
    return cf.reshape(ic, oc, P, 32)


def _prep_host(x, twiddle, bias):
    """All host-side layout work. Returns per-core input maps (numpy)."""
    wnp = _NPDT[W_DT]
    ynp = _NPDT[Y_DT]
    perm = _bitrev(N)
    y = np.ascontiguousarray(x).reshape(IC, B, N)[:, :, perm]
    # device layout y[ic, p, c*64+b]
    y_dev = np.ascontiguousarray(
        y.reshape(IC, B, NCH, P).transpose(0, 3, 2, 1)
    ).reshape(IC, P, NCH * B).astype(ynp)

    A = _compose_stageA(np.asarray(twiddle, dtype=np.float32))
    cf = _stageB_coeffs(np.asarray(twiddle, dtype=np.float32))

    # bias in device (b,cp) layout: bias_dev[oc, p, b*8+cp] = bias[oc, cp*128+p]
    bias_pc = np.asarray(bias, dtype=np.float32).reshape(OC, NCH, P)
    bias_dev = np.broadcast_to(
        bias_pc.transpose(0, 2, 1)[:, :, None, :], (OC, P, B, NCH)
    ).reshape(OC, P, NCH * B)

    ident = np.eye(P, dtype=np.float32).astype(ml_dtypes.bfloat16)

    in_maps = []
    for core in range(NCORES):
        osl = slice(core * OCL, (core + 1) * OCL)
        Ac = A[:, osl]  # (IC, OCL, 4, 256, 256)
        # lhsT tiles: w[ic,o,p_k, g, h, kin, m] = Ac[ic,o,g][h*128+m, kin*128+p_k]
        w = np.ascontiguousarray(
            Ac.reshape(IC, OCL, NBLK, 2, P, 2, P)  # [g, h, m, kin, k]
            .transpose(0, 1, 6, 2, 3, 5, 4)        # [ic,o,k,g,h,kin,m]
        ).astype(wnp)
        in_maps.append(
            {
                "y": y_dev,
                "w": w.reshape(IC, OCL, P, NBLK * 4 * P),
                "cf": np.ascontiguousarray(cf[:, osl]).astype(ml_dtypes.bfloat16),
                "bias": np.ascontiguousarray(bias_dev[osl]),
                "ident": ident,
            }
        )
    return in_maps


def _emulate_core(im):
    """Numpy emulation of the device program (for validating layout math)."""
    y = im["y"].astype(np.float32)      # (IC, 128, 512) free=(cp,b)
    w = im["w"].astype(np.float32).reshape(IC, OCL, P, NBLK, 2, 2, P)
    cf = im["cf"].astype(np.float32).reshape(IC, OCL, P, 4, 8)
    out = np.array(im["bias"], dtype=np.float32).reshape(OCL, P, B, NCH).copy()
    bf = lambda a: a.astype(ml_dtypes.bfloat16).astype(np.float32)
    for o in range(OCL):
        acc = np.zeros((P, B, NCH), dtype=np.float32)
        for ic in range(IC):
            yv = y[ic].reshape(P, NCH, B)
            z = np.zeros((P, NCH, B), dtype=np.float32)
            for g in range(NBLK):
                for h in range(2):
                    a = np.zeros((P, B), dtype=np.float32)
                    for kin in range(2):
                        lhsT = w[ic, o, :, g, h, kin]  # [k, m]
                        a += lhsT.T @ yv[:, 2 * g + kin]
                    z[:, 2 * g + h] = a
            # Act transpose-convert -> zb[p, b, n9, n8, n7] bf16
            zb = bf(z.reshape(P, 2, 2, 2, B).transpose(0, 4, 1, 2, 3))
            c = cf[ic, o].reshape(P, 1, 4, 2, 2, 2)
            c8_0 = c[:, :, 0].transpose(0, 1, 3, 2, 4)  # (n8',n9,n7)->(n9,n8',n7)
            c8_1 = c[:, :, 1].transpose(0, 1, 3, 2, 4)
            u0 = bf(zb[:, :, :, 0:1, :] * c8_0)
            u1 = bf(zb[:, :, :, 1:2, :] * c8_1)
            y8 = bf(u0 + u1)                          # [p, b, n9, n8', n7]
            v0 = bf(y8[:, :, 0:1] * c[:, :, 2])
            v1 = bf(y8[:, :, 1:2] * c[:, :, 3])
            acc += (v0 + v1).reshape(P, B, NCH)
        out[o] += acc
    return out.reshape(OCL, P, NCH * B)


def _build_program():
    import concourse.bacc as bacc
    import concourse.mybir as mybir
    from concourse.tile import TileContext

    f32 = mybir.dt.float32
    bf16 = mybir.dt.bfloat16
    wdt = mybir.dt.float8e4 if W_DT == "fp8" else mybir.dt.bfloat16
    ydt = mybir.dt.float8e4 if Y_DT == "fp8" else mybir.dt.bfloat16
    MULT, ADD = mybir.AluOpType.mult, mybir.AluOpType.add
    COPY = mybir.ActivationFunctionType.Copy

    nc = bacc.Bacc(None, target_bir_lowering=False)
    y_d = nc.dram_tensor("y", (IC, P, NCH * B), ydt, kind="ExternalInput")
    w_d = nc.dram_tensor("w", (IC, OCL, P, NBLK * 4 * P), wdt, kind="ExternalInput")
    cf_d = nc.dram_tensor("cf", (IC, OCL, P, 32), bf16, kind="ExternalInput")
    bias_d = nc.dram_tensor("bias", (OCL, P, NCH * B), f32, kind="ExternalInput")
    id_d = nc.dram_tensor("ident", (P, P), bf16, kind="ExternalInput")
    o_d = nc.dram_tensor("o", (OCL, P, NCH * B), f32, kind="ExternalOutput")

    with TileContext(nc) as tc:
        with (
            tc.tile_pool(name="ypool", bufs=2) as ypool,
            tc.tile_pool(name="wpool", bufs=3) as wpool,
            tc.tile_pool(name="cfpool", bufs=3) as cfpool,
            tc.tile_pool(name="zbpool", bufs=2) as zbpool,
            tc.tile_pool(name="upool", bufs=2) as upool,
            tc.tile_pool(name="vpool", bufs=2) as vpool,
            tc.tile_pool(name="misc", bufs=1) as misc,
            tc.tile_pool(name="zpsum", bufs=3, space="PSUM") as zpsum,
            tc.tile_pool(name="apsum", bufs=OCL, space="PSUM") as apsum,
        ):
            ident = misc.tile([P, P], bf16, tag="ident")
            nc.sync.dma_start(out=ident[:], in_=id_d[:, :])
            accs = []
            for o in range(OCL):
                acc = apsum.tile([P, NCH * B], f32, tag="acc")
                accs.append(acc)

            pend = None  # deferred acc-matmuls: (o, v0, v1, first)

            def flush_pend():
                nonlocal pend
                if pend is None:
                    return
                o, v0, v1, first, last = pend
                nc.tensor.matmul(
                    accs[o][:],
                    ident[:],
                    v0[:].rearrange("p b x y z -> p (b x y z)"),
                    start=first, stop=False, skip_group_check=True,
                )
                nc.tensor.matmul(
                    accs[o][:],
                    ident[:],
                    v1[:].rearrange("p b x y z -> p (b x y z)"),
                    start=False, stop=last, skip_group_check=True,
                )
                pend = None

            for ic in range(IC):
                ytile = ypool.tile([P, NCH * B], ydt)
                nc.sync.dma_start(out=ytile[:], in_=y_d[ic])
                for o in range(OCL):
                    wtile = wpool.tile([P, NBLK * 4 * P], wdt)
                    nc.sync.dma_start(out=wtile[:], in_=w_d[ic, o])
                    cft = cfpool.tile([P, 4, 8], bf16)
                    nc.sync.dma_start(
                        out=cft[:], in_=cf_d[ic, o].rearrange("p (j c) -> p j c", j=4)
                    )
                    z = zpsum.tile([P, NCH * B], f32)
                    for g in range(NBLK):
                        for h in range(2):
                            cp = 2 * g + h
                            for kin in range(2):
                                wi = ((g * 2 + h) * 2 + kin) * P
                                nc.tensor.matmul(
                                    z[:, cp * B : (cp + 1) * B],
                                    wtile[:, wi : wi + P],
                                    ytile[:, (2 * g + kin) * B : (2 * g + kin + 1) * B],
                                    start=(kin == 0),
                                    stop=(kin == 1),
                                )
                    # deferred acc-MMs of the PREVIOUS pair go after this
                    # pair's z-MMs so PE never waits on DVE head-of-line.
                    flush_pend()

                    # Act: transpose-convert z (cp,b) f32 -> zb (b,n9,n8,n7) bf16.
                    # Strided INPUT AP (free on Act), contiguous packed output.
                    zb = zbpool.tile([P, B, 2, 2, 2], bf16)
                    z_bc = z[:].rearrange("p (c b) -> p b c", c=NCH)
                    nc.scalar.activation(
                        zb[:].rearrange("p b x y z -> p b (x y z)"),
                        z_bc, COPY, scale=1.0,
                    )

                    # Stage B: tensor_tensor products on DVE (2x bf16 mode on
                    # HW; stt does NOT accelerate). HW AP limit is 2 free dims
                    # per operand, so each product op splits over the bit that
                    # is broadcast in its gather operand (2 ops of 256 els).
                    # The u0+u1 add runs on the otherwise-idle Pool engine.
                    cg = cft[:].rearrange("p j (x y z) -> p j x y z", x=2, y=2)
                    u0 = upool.tile([P, B, 2, 2, 2], bf16, tag="u0")
                    u1 = upool.tile([P, B, 2, 2, 2], bf16, tag="u1")
                    for q, ut in ((0, u0), (1, u1)):
                        # u_q[p,b,n9,n8o,n7] = zb[p,b,n9,q,n7] * c8q[p,n8o,n9,n7]
                        zg = zb[:, :, :, q, :]                  # [P,B,2,2]
                        for n8o in range(2):
                            cq = cg[:, q, n8o].unsqueeze(1).broadcast_to(
                                (P, B, 2, 2)
                            )
                            nc.vector.tensor_tensor(
                                ut[:, :, :, n8o, :], zg, cq, MULT
                            )
                    y8 = upool.tile([P, B, 2, 2, 2], bf16, tag="y8")
                    nc.gpsimd.tensor_tensor(
                        y8[:].rearrange("p b x y z -> p (b x y z)"),
                        u0[:].rearrange("p b x y z -> p (b x y z)"),
                        u1[:].rearrange("p b x y z -> p (b x y z)"),
                        ADD,
                    )
                    v0 = vpool.tile([P, B, 2, 2, 2], bf16, tag="v0")
                    v1 = vpool.tile([P, B, 2, 2, 2], bf16, tag="v1")
                    for q, vt in ((0, v0), (1, v1)):
                        # v_q[p,b,n9o,n8o,n7] = y8[p,b,q,n8o,n7] * c9q[p,n9o,n8o,n7]
                        yg = y8[:, :, q, :, :]                  # [P,B,2,2]
                        for n9o in range(2):
                            cq = cg[:, 2 + q, n9o].unsqueeze(1).broadcast_to(
                                (P, B, 2, 2)
                            )
                            nc.vector.tensor_tensor(
                                vt[:, :, n9o, :, :], yg, cq, MULT
                            )
                    pend = (o, v0, v1, ic == 0, ic == IC - 1)
            flush_pend()

            for o in range(OCL):
                biast = misc.tile([P, NCH * B], f32, tag=f"bias{o}")
                nc.sync.dma_start(out=biast[:], in_=bias_d[o])
                outt = misc.tile([P, NCH * B], f32, tag=f"out{o}")
                nc.vector.scalar_tensor_tensor(
                    outt[:], accs[o][:], 1.0, biast[:], MULT, ADD
                )
                nc.sync.dma_start(out=o_d[o], in_=outt[:])
    nc.finalize()
    return nc


_LAST_RESULTS = {"exec_time_ns": None}


def kernel(x, twiddle, bias, _trace=False, _emulate=False):
    in_maps = _prep_host(np.asarray(x), np.asarray(twiddle), np.asarray(bias))
    if _emulate:
        outs = [_emulate_core(im) for im in in_maps]
    else:
        from concourse.bass_utils import run_bass_kernel_spmd

        nc = _build_program()
        res = run_bass_kernel_spmd(nc, in_maps, list(range(NCORES)), trace=_trace)
        _LAST_RESULTS["exec_time_ns"] = res.exec_time_ns
        _LAST_RESULTS["mean_exec_time_ns"] = res.mean_exec_time_ns
        outs = [r["o"] for r in res.results]
    # o[oc_l, p, b*8+cp] -> (OC, B, N) with n = cp*128+p; final (B,OC,H,W)
    # is a pure reinterpret of (OC,B,N) bytes (reference uses .reshape).
    full = np.concatenate(
        [
            np.asarray(o, dtype=np.float32)
            .reshape(OCL, P, B, NCH)
            .transpose(0, 2, 3, 1)
            .reshape(OCL, B, N)
            for o in outs
        ],
        axis=0,
    )
    return np.ascontiguousarray(full).reshape(B, OC, H, W).astype(np.float32)


# revision 13
# speedup vs baseline: 2.6144x; 1.0241x over previous
"""Butterfly-Conv2d (nn_BConv2d) Trainium2 kernel — v2 (low-precision, PE-accumulated).

Math (reference): x(B=64,IC=16,32,32) -> y=x.reshape(IC,B,N=1024)[:,:,bitrev];
broadcast over OC=32; 10 radix-2 butterfly layers with per-(ic,oc) twiddles;
mean over ic; + bias -> (B,OC,32,32).

Strategy (per core: all 16 ic x 4 oc, core-local ic-mean, no collective):
  * Host (free): compose butterfly layers 0..7 into dense 256x256 blocks
    (4 per (ic,oc)); cast weights to fp8/bf16. Build per-(p,chunk) coeff
    vectors for layers 8/9 (1/16 mean folded into layer 9).
  * Stage A (PE): 16 matmuls per (ic,oc) [k=128, m=128, free=64] with PSUM
    K-accumulation -> z[p, (cp,b)] f32 in PSUM.
  * Act: transpose-convert z -> SBUF bf16 in (b, n9, n8, n7) free layout
    (coefficient broadcasts then have packed last dims -> DVE 4x mode).
  * Stage B (DVE): 5 wide scalar_tensor_tensor ops per (ic,oc):
      u_q = z[b, n9, q, n7] * c8_q[n9, n8', n7]   (q=0,1; broadcast over b)
      y8  = u0 + u1
      v_q = y8[b, q, n8', n7] * c9_q[n9', n8', n7]
    (stt with immediate scalar 1.0 => InstTensorScalarPtr, 4x perf mode)
  * ic-mean accumulation (PE): acc_o += I @ v_q, 32 accumulating matmuls
    into a persistent PSUM bank per oc.
  * Epilogue: out_o = acc_o + bias (DVE), DMA out.

Device output layout: o[ocl, p, b*8+cp] with n = cp*128+p.
"""

import numpy as np
import ml_dtypes

B, IC, OC, H, W = 64, 16, 32, 32, 32
N = H * W          # 1024
M = 10             # butterfly layers
NCORES = 8
OCL = OC // NCORES  # 4 oc per core
NCH = 8            # free-dim chunks (n9n8n7)
P = 128            # partitions (n6..n0)
SB = 256           # composed stage-A block size (layers 0..7)
NBLK = N // SB     # 4 blocks per (ic,oc)

# stage-A weight dtype: "fp8" (float8_e4m3) or "bf16"
W_DT = "bf16"
Y_DT = "bf16"      # stage-A rhs dtype (mixed fp8 lhsT x bf16 rhs verified on HW)

_NPDT = {"fp8": ml_dtypes.float8_e4m3, "bf16": ml_dtypes.bfloat16}


def _bitrev(n):
    bits = int(np.log2(n))
    idx = np.arange(n, dtype=np.int64)
    rev = np.zeros(n, dtype=np.int64)
    for b in range(bits):
        rev = (rev << 1) | ((idx >> b) & 1)
    return rev


def _compose_stageA(tw):
    """Compose butterfly layers 0..7 into A[ic,oc,g,256,256] (g=4 blocks)."""
    ic, oc = tw.shape[0], tw.shape[1]
    A = np.zeros((ic, oc, NBLK, SB, SB), dtype=np.float32)
    eye = np.eye(SB, dtype=np.float32)
    A[:] = eye
    for l in range(8):
        s = 1 << l
        nb_loc = SB // (2 * s)
        t = tw[:, :, l].reshape(ic, oc, N // (2 * s), s, 2, 2)
        t = t.reshape(ic, oc, NBLK, nb_loc, s, 2, 2)
        Av = A.reshape(ic, oc, NBLK, nb_loc, 2, s, SB)
        a0 = Av[:, :, :, :, 0]
        a1 = Av[:, :, :, :, 1]
        t00 = t[..., 0, 0, None]
        t01 = t[..., 0, 1, None]
        t10 = t[..., 1, 0, None]
        t11 = t[..., 1, 1, None]
        new0 = t00 * a0 + t01 * a1
        new1 = t10 * a0 + t11 * a1
        Av[:, :, :, :, 0] = new0
        Av[:, :, :, :, 1] = new1
    return A


def _stageB_coeffs(tw):
    """cf[ic, oc, p, 32] f32: 4 groups of 8 chunk-coeffs per partition.

    group j=0/1: layer-8 q=0/1 coeff, stored n8'-major (n8', n9, n7) so the
        per-n8' slice is contiguous (HW AP limit: 2 free dims/operand):
        cf[.., q, n8o*4+n9*2+n7] = t8[k=n9, n7*128+p, n8', q]
    group j=2/3: layer-9 q9=0/1 coeff, stored n9'-major (n9', n8', n7):
        cf[.., 2+q, n9o*4+n8o*2+n7] = t9[0, n8'*256+n7*128+p, n9', q9] / IC
    """
    ic, oc = tw.shape[0], tw.shape[1]
    t8 = tw[:, :, 8].reshape(ic, oc, 2, 256, 2, 2)   # [k, j, p_out, q]
    t9 = tw[:, :, 9].reshape(ic, oc, 512, 2, 2)      # [j, p_out, q]
    cf = np.zeros((ic, oc, P, 4, 8), dtype=np.float32)
    pr = np.arange(P)
    for n9 in range(2):
        for n8o in range(2):
            for n7 in range(2):
                for q in range(2):
                    cf[:, :, :, q, n8o * 4 + n9 * 2 + n7] = t8[
                        :, :, n9, n7 * 128 + pr, n8o, q
                    ]
                    cf[:, :, :, 2 + q, n9 * 4 + n8o * 2 + n7] = (
                        t9[:, :, n8o * 256 + n7 * 128 + pr, n9, q] / IC
                    )
    return cf.reshape(ic, oc, P, 32)


def _prep_host(x, twiddle, bias):
    """All host-side layout work. Returns per-core input maps (numpy)."""
    wnp = _NPDT[W_DT]
    ynp = _NPDT[Y_DT]
    perm = _bitrev(N)
    y = np.ascontiguousarray(x).reshape(IC, B, N)[:, :, perm]
    # device layout y[ic, p, c*64+b]
    y_dev = np.ascontiguousarray(
        y.reshape(IC, B, NCH, P).transpose(0, 3, 2, 1)
    ).reshape(IC, P, NCH * B).astype(ynp)

    A = _compose_stageA(np.asarray(twiddle, dtype=np.float32))
    cf = _stageB_coeffs(np.asarray(twiddle, dtype=np.float32))

    # bias in device (b,cp) layout: bias_dev[oc, p, b*8+cp] = bias[oc, cp*128+p]
    bias_pc = np.asarray(bias, dtype=np.float32).reshape(OC, NCH, P)
    bias_dev = np.broadcast_to(
        bias_pc.transpose(0, 2, 1)[:, :, None, :], (OC, P, B, NCH)
    ).reshape(OC, P, NCH * B)

    ident = np.eye(P, dtype=np.float32).astype(ml_dtypes.bfloat16)

    in_maps = []
    for core in range(NCORES):
        osl = slice(core * OCL, (core + 1) * OCL)
        Ac = A[:, osl]  # (IC, OCL, 4, 256, 256)
        # lhsT tiles: w[ic,o,p_k, g, h, kin, m] = Ac[ic,o,g][h*128+m, kin*128+p_k]
        w = np.ascontiguousarray(
            Ac.reshape(IC, OCL, NBLK, 2, P, 2, P)  # [g, h, m, kin, k]
            .transpose(0, 1, 6, 2, 3, 5, 4)        # [ic,o,k,g,h,kin,m]
        ).astype(wnp)
        in_maps.append(
            {
                "y": y_dev,
                "w": w.reshape(IC, OCL, P, NBLK * 4 * P),
                "cf": np.ascontiguousarray(cf[:, osl]).astype(ml_dtypes.bfloat16),
                "bias": np.ascontiguousarray(bias_dev[osl]),
                "ident": ident,
            }
        )
    return in_maps


def _emulate_core(im):
    """Numpy emulation of the device program (for validating layout math)."""
    y = im["y"].astype(np.float32)      # (IC, 128, 512) free=(cp,b)
    w = im["w"].astype(np.float32).reshape(IC, OCL, P, NBLK, 2, 2, P)
    cf = im["cf"].astype(np.float32).reshape(IC, OCL, P, 4, 8)
    out = np.array(im["bias"], dtype=np.float32).reshape(OCL, P, B, NCH).copy()
    bf = lambda a: a.astype(ml_dtypes.bfloat16).astype(np.float32)
    for o in range(OCL):
        acc = np.zeros((P, B, NCH), dtype=np.float32)
        for ic in range(IC):
            yv = y[ic].reshape(P, NCH, B)
            z = np.zeros((P, NCH, B), dtype=np.float32)
            for g in range(NBLK):
                for h in range(2):
                    a = np.zeros((P, B), dtype=np.float32)
                    for kin in range(2):
                        lhsT = w[ic, o, :, g, h, kin]  # [k, m]
                        a += lhsT.T @ yv[:, 2 * g + kin]
                    z[:, 2 * g + h] = a
            # Act transpose-convert -> zb[p, b, n9, n8, n7] bf16
            zb = bf(z.reshape(P, 2, 2, 2, B).transpose(0, 4, 1, 2, 3))
            c = cf[ic, o].reshape(P, 1, 4, 2, 2, 2)
            c8_0 = c[:, :, 0].transpose(0, 1, 3, 2, 4)  # (n8',n9,n7)->(n9,n8',n7)
            c8_1 = c[:, :, 1].transpose(0, 1, 3, 2, 4)
            u0 = bf(zb[:, :, :, 0:1, :] * c8_0)
            u1 = bf(zb[:, :, :, 1:2, :] * c8_1)
            y8 = bf(u0 + u1)                          # [p, b, n9, n8', n7]
            v0 = bf(y8[:, :, 0:1] * c[:, :, 2])
            v1 = bf(y8[:, :, 1:2] * c[:, :, 3])
            acc += (v0 + v1).reshape(P, B, NCH)
        out[o] += acc
    return out.reshape(OCL, P, NCH * B)


def _build_program():
    import concourse.bacc as bacc
    import concourse.mybir as mybir
    from concourse.tile import TileContext

    f32 = mybir.dt.float32
    bf16 = mybir.dt.bfloat16
    wdt = mybir.dt.float8e4 if W_DT == "fp8" else mybir.dt.bfloat16
    ydt = mybir.dt.float8e4 if Y_DT == "fp8" else mybir.dt.bfloat16
    MULT, ADD = mybir.AluOpType.mult, mybir.AluOpType.add
    COPY = mybir.ActivationFunctionType.Copy

    nc = bacc.Bacc(None, target_bir_lowering=False)
    y_d = nc.dram_tensor("y", (IC, P, NCH * B), ydt, kind="ExternalInput")
    w_d = nc.dram_tensor("w", (IC, OCL, P, NBLK * 4 * P), wdt, kind="ExternalInput")
    cf_d = nc.dram_tensor("cf", (IC, OCL, P, 32), bf16, kind="ExternalInput")
    bias_d = nc.dram_tensor("bias", (OCL, P, NCH * B), f32, kind="ExternalInput")
    id_d = nc.dram_tensor("ident", (P, P), bf16, kind="ExternalInput")
    o_d = nc.dram_tensor("o", (OCL, P, NCH * B), f32, kind="ExternalOutput")

    with TileContext(nc) as tc:
        with (
            tc.tile_pool(name="ypool", bufs=2) as ypool,
            tc.tile_pool(name="wpool", bufs=3) as wpool,
            tc.tile_pool(name="cfpool", bufs=3) as cfpool,
            tc.tile_pool(name="zbpool", bufs=2) as zbpool,
            tc.tile_pool(name="upool", bufs=2) as upool,
            tc.tile_pool(name="vpool", bufs=2) as vpool,
            tc.tile_pool(name="misc", bufs=1) as misc,
            tc.tile_pool(name="zpsum", bufs=3, space="PSUM") as zpsum,
            tc.tile_pool(name="apsum", bufs=OCL, space="PSUM") as apsum,
        ):
            ident = misc.tile([P, P], bf16, tag="ident")
            nc.sync.dma_start(out=ident[:], in_=id_d[:, :])
            accs = []
            for o in range(OCL):
                acc = apsum.tile([P, NCH * B], f32, tag="acc")
                accs.append(acc)

            # Software pipelining: pair i's v-products (which depend on the
            # Pool y8-add) and acc-matmuls are emitted during pair i+1, so
            # DVE's in-order stream never head-of-line blocks on Pool, and PE
            # never waits on DVE before the next pair's z-matmuls.
            pend = None  # (o, y8, cg, first, last) of the previous pair

            def flush_pend():
                nonlocal pend
                if pend is None:
                    return
                o, y8, cg, first, last = pend
                vts = []
                for q in range(2):
                    vt = vpool.tile([P, B, 2, 2, 2], bf16, tag=f"v{q}",
                                    name=f"v{q}")
                    # v_q[p,b,n9o,n8o,n7] = y8[p,b,q,n8o,n7] * c9q[p,n9o,n8o,n7]
                    yg = y8[:, :, q, :, :]                  # [P,B,2,2]
                    for n9o in range(2):
                        cq = cg[:, 2 + q, n9o].unsqueeze(1).broadcast_to(
                            (P, B, 2, 2)
                        )
                        nc.vector.tensor_tensor(
                            vt[:, :, n9o, :, :], yg, cq, MULT
                        )
                    vts.append(vt)
                for q, vt in enumerate(vts):
                    nc.tensor.matmul(
                        accs[o][:],
                        ident[:],
                        vt[:].rearrange("p b x y z -> p (b x y z)"),
                        start=(first and q == 0), stop=(last and q == 1),
                        skip_group_check=True,
                    )
                pend = None

            for ic in range(IC):
                ytile = ypool.tile([P, NCH * B], ydt)
                nc.sync.dma_start(out=ytile[:], in_=y_d[ic])
                for o in range(OCL):
                    wtile = wpool.tile([P, NBLK * 4 * P], wdt)
                    nc.sync.dma_start(out=wtile[:], in_=w_d[ic, o])
                    cft = cfpool.tile([P, 4, 8], bf16)
                    nc.sync.dma_start(
                        out=cft[:], in_=cf_d[ic, o].rearrange("p (j c) -> p j c", j=4)
                    )
                    z = zpsum.tile([P, NCH * B], f32)
                    for g in range(NBLK):
                        for h in range(2):
                            cp = 2 * g + h
                            for kin in range(2):
                                wi = ((g * 2 + h) * 2 + kin) * P
                                nc.tensor.matmul(
                                    z[:, cp * B : (cp + 1) * B],
                                    wtile[:, wi : wi + P],
                                    ytile[:, (2 * g + kin) * B : (2 * g + kin + 1) * B],
                                    start=(kin == 0),
                                    stop=(kin == 1),
                                )
                    # Act: transpose-convert z (cp,b) f32 -> zb (b,n9,n8,n7) bf16.
                    # Strided INPUT AP (free on Act), contiguous packed output.
                    zb = zbpool.tile([P, B, 2, 2, 2], bf16)
                    z_bc = z[:].rearrange("p (c b) -> p b c", c=NCH)
                    nc.scalar.activation(
                        zb[:].rearrange("p b x y z -> p b (x y z)"),
                        z_bc, COPY, scale=1.0,
                    )

                    # Stage B: tensor_tensor products on DVE (2x bf16 mode on
                    # HW; stt does NOT accelerate). HW AP limit is 2 free dims
                    # per operand, so each product op splits over the bit that
                    # is broadcast in its gather operand (2 ops of 256 els).
                    # The u0+u1 add runs on the otherwise-idle Pool engine.
                    cg = cft[:].rearrange("p j (x y z) -> p j x y z", x=2, y=2)
                    u0 = upool.tile([P, B, 2, 2, 2], bf16, tag="u0")
                    u1 = upool.tile([P, B, 2, 2, 2], bf16, tag="u1")
                    for q, ut in ((0, u0), (1, u1)):
                        # u_q[p,b,n9,n8o,n7] = zb[p,b,n9,q,n7] * c8q[p,n8o,n9,n7]
                        zg = zb[:, :, :, q, :]                  # [P,B,2,2]
                        for n8o in range(2):
                            cq = cg[:, q, n8o].unsqueeze(1).broadcast_to(
                                (P, B, 2, 2)
                            )
                            nc.vector.tensor_tensor(
                                ut[:, :, :, n8o, :], zg, cq, MULT
                            )
                    y8 = upool.tile([P, B, 2, 2, 2], bf16, tag="y8")
                    nc.gpsimd.tensor_tensor(
                        y8[:].rearrange("p b x y z -> p (b x y z)"),
                        u0[:].rearrange("p b x y z -> p (b x y z)"),
                        u1[:].rearrange("p b x y z -> p (b x y z)"),
                        ADD,
                    )
                    # deferred stage of the PREVIOUS pair
                    flush_pend()
                    pend = (o, y8, cg, ic == 0, ic == IC - 1)
            flush_pend()

            for o in range(OCL):
                biast = misc.tile([P, NCH * B], f32, tag=f"bias{o}")
                nc.sync.dma_start(out=biast[:], in_=bias_d[o])
                outt = misc.tile([P, NCH * B], f32, tag=f"out{o}")
                nc.vector.scalar_tensor_tensor(
                    outt[:], accs[o][:], 1.0, biast[:], MULT, ADD
                )
                nc.sync.dma_start(out=o_d[o], in_=outt[:])
    nc.finalize()
    return nc


_LAST_RESULTS = {"exec_time_ns": None}


def kernel(x, twiddle, bias, _trace=False, _emulate=False):
    in_maps = _prep_host(np.asarray(x), np.asarray(twiddle), np.asarray(bias))
    if _emulate:
        outs = [_emulate_core(im) for im in in_maps]
    else:
        from concourse.bass_utils import run_bass_kernel_spmd

        nc = _build_program()
        res = run_bass_kernel_spmd(nc, in_maps, list(range(NCORES)), trace=_trace)
        _LAST_RESULTS["exec_time_ns"] = res.exec_time_ns
        _LAST_RESULTS["mean_exec_time_ns"] = res.mean_exec_time_ns
        outs = [r["o"] for r in res.results]
    # o[oc_l, p, b*8+cp] -> (OC, B, N) with n = cp*128+p; final (B,OC,H,W)
    # is a pure reinterpret of (OC,B,N) bytes (reference uses .reshape).
    full = np.concatenate(
        [
            np.asarray(o, dtype=np.float32)
            .reshape(OCL, P, B, NCH)
            .transpose(0, 2, 3, 1)
            .reshape(OCL, B, N)
            for o in outs
        ],
        axis=0,
    )
    return np.ascontiguousarray(full).reshape(B, OC, H, W).astype(np.float32)


# revision 24
# speedup vs baseline: 2.7741x; 1.0611x over previous
"""Butterfly-Conv2d (nn_BConv2d) Trainium2 kernel — v2 (low-precision, PE-accumulated).

Math (reference): x(B=64,IC=16,32,32) -> y=x.reshape(IC,B,N=1024)[:,:,bitrev];
broadcast over OC=32; 10 radix-2 butterfly layers with per-(ic,oc) twiddles;
mean over ic; + bias -> (B,OC,32,32).

Strategy (per core: all 16 ic x 4 oc, core-local ic-mean, no collective):
  * Host (free): compose butterfly layers 0..7 into dense 256x256 blocks
    (4 per (ic,oc)); cast weights to fp8/bf16. Build per-(p,chunk) coeff
    vectors for layers 8/9 (1/16 mean folded into layer 9).
  * Stage A (PE): 16 matmuls per (ic,oc) [k=128, m=128, free=64] with PSUM
    K-accumulation -> z[p, (cp,b)] f32 in PSUM.
  * Act: transpose-convert z -> SBUF bf16 in (b, n9, n8, n7) free layout
    (coefficient broadcasts then have packed last dims -> DVE 4x mode).
  * Stage B (DVE): 5 wide scalar_tensor_tensor ops per (ic,oc):
      u_q = z[b, n9, q, n7] * c8_q[n9, n8', n7]   (q=0,1; broadcast over b)
      y8  = u0 + u1
      v_q = y8[b, q, n8', n7] * c9_q[n9', n8', n7]
    (stt with immediate scalar 1.0 => InstTensorScalarPtr, 4x perf mode)
  * ic-mean accumulation (PE): acc_o += I @ v_q, 32 accumulating matmuls
    into a persistent PSUM bank per oc.
  * Epilogue: out_o = acc_o + bias (DVE), DMA out.

Device output layout: o[ocl, p, b*8+cp] with n = cp*128+p.
"""

import numpy as np
import ml_dtypes

B, IC, OC, H, W = 64, 16, 32, 32, 32
N = H * W          # 1024
M = 10             # butterfly layers
NCORES = 8
OCL = OC // NCORES  # 4 oc per core
NCH = 8            # free-dim chunks (n9n8n7)
P = 128            # partitions (n6..n0)
SB = 256           # composed stage-A block size (layers 0..7)
NBLK = N // SB     # 4 blocks per (ic,oc)

# stage-A weight dtype: "fp8e3" (float8_e3m4, per-pair scaled), "fp8"
# (float8_e4m3) or "bf16"
W_DT = "fp8e3"
Y_DT = "bf16"      # stage-A rhs dtype (mixed fp8 lhsT x bf16 rhs verified on HW)

_NPDT = {
    "fp8": ml_dtypes.float8_e4m3,
    "fp8e3": ml_dtypes.float8_e3m4,
    "bf16": ml_dtypes.bfloat16,
}


def _bitrev(n):
    bits = int(np.log2(n))
    idx = np.arange(n, dtype=np.int64)
    rev = np.zeros(n, dtype=np.int64)
    for b in range(bits):
        rev = (rev << 1) | ((idx >> b) & 1)
    return rev


def _compose_stageA(tw):
    """Compose butterfly layers 0..7 into A[ic,oc,g,256,256] (g=4 blocks)."""
    ic, oc = tw.shape[0], tw.shape[1]
    A = np.zeros((ic, oc, NBLK, SB, SB), dtype=np.float32)
    eye = np.eye(SB, dtype=np.float32)
    A[:] = eye
    for l in range(8):
        s = 1 << l
        nb_loc = SB // (2 * s)
        t = tw[:, :, l].reshape(ic, oc, N // (2 * s), s, 2, 2)
        t = t.reshape(ic, oc, NBLK, nb_loc, s, 2, 2)
        Av = A.reshape(ic, oc, NBLK, nb_loc, 2, s, SB)
        a0 = Av[:, :, :, :, 0]
        a1 = Av[:, :, :, :, 1]
        t00 = t[..., 0, 0, None]
        t01 = t[..., 0, 1, None]
        t10 = t[..., 1, 0, None]
        t11 = t[..., 1, 1, None]
        new0 = t00 * a0 + t01 * a1
        new1 = t10 * a0 + t11 * a1
        Av[:, :, :, :, 0] = new0
        Av[:, :, :, :, 1] = new1
    return A


def _stageB_coeffs(tw):
    """cf[ic, oc, p, 32] f32: 4 groups of 8 chunk-coeffs per partition.

    group j=0/1: layer-8 q=0/1 coeff, stored n8'-major (n8', n9, n7) so the
        per-n8' slice is contiguous (HW AP limit: 2 free dims/operand):
        cf[.., q, n8o*4+n9*2+n7] = t8[k=n9, n7*128+p, n8', q]
    group j=2/3: layer-9 q9=0/1 coeff, stored n9'-major (n9', n8', n7):
        cf[.., 2+q, n9o*4+n8o*2+n7] = t9[0, n8'*256+n7*128+p, n9', q9] / IC
    """
    ic, oc = tw.shape[0], tw.shape[1]
    t8 = tw[:, :, 8].reshape(ic, oc, 2, 256, 2, 2)   # [k, j, p_out, q]
    t9 = tw[:, :, 9].reshape(ic, oc, 512, 2, 2)      # [j, p_out, q]
    cf = np.zeros((ic, oc, P, 4, 8), dtype=np.float32)
    pr = np.arange(P)
    for n9 in range(2):
        for n8o in range(2):
            for n7 in range(2):
                for q in range(2):
                    cf[:, :, :, q, n8o * 4 + n9 * 2 + n7] = t8[
                        :, :, n9, n7 * 128 + pr, n8o, q
                    ]
                    cf[:, :, :, 2 + q, n9 * 4 + n8o * 2 + n7] = (
                        t9[:, :, n8o * 256 + n7 * 128 + pr, n9, q] / IC
                    )
    return cf.reshape(ic, oc, P, 32)


def _prep_host(x, twiddle, bias):
    """All host-side layout work. Returns per-core input maps (numpy)."""
    wnp = _NPDT[W_DT]
    ynp = _NPDT[Y_DT]
    perm = _bitrev(N)
    y = np.ascontiguousarray(x).reshape(IC, B, N)[:, :, perm]
    # device layout y[ic, p, c*64+b]
    y_dev = np.ascontiguousarray(
        y.reshape(IC, B, NCH, P).transpose(0, 3, 2, 1)
    ).reshape(IC, P, NCH * B).astype(ynp)

    A = _compose_stageA(np.asarray(twiddle, dtype=np.float32))
    cf = _stageB_coeffs(np.asarray(twiddle, dtype=np.float32))
    if W_DT == "fp8e3":
        # per-(ic,oc) scale: use e3m4's range, fold 1/s into layer-8 coeffs
        amax = np.abs(A).max(axis=(2, 3, 4))          # (IC, OC)
        s = 8.0 / np.maximum(amax, 1e-30)
        A = A * s[:, :, None, None, None]
        cfv = cf.reshape(IC, OC, P, 4, 8)
        cfv[:, :, :, 0:2] /= s[:, :, None, None, None]

    # bias in device (b,cp) layout: bias_dev[oc, p, b*8+cp] = bias[oc, cp*128+p]
    bias_pc = np.asarray(bias, dtype=np.float32).reshape(OC, NCH, P)
    bias_dev = np.broadcast_to(
        bias_pc.transpose(0, 2, 1)[:, :, None, :], (OC, P, B, NCH)
    ).reshape(OC, P, NCH * B)

    ident = np.eye(P, dtype=np.float32).astype(ml_dtypes.bfloat16)

    in_maps = []
    for core in range(NCORES):
        osl = slice(core * OCL, (core + 1) * OCL)
        Ac = A[:, osl]  # (IC, OCL, 4, 256, 256)
        # lhsT tiles: w[ic,o,p_k, g, h, kin, m] = Ac[ic,o,g][h*128+m, kin*128+p_k]
        w = np.ascontiguousarray(
            Ac.reshape(IC, OCL, NBLK, 2, P, 2, P)  # [g, h, m, kin, k]
            .transpose(0, 1, 6, 2, 3, 5, 4)        # [ic,o,k,g,h,kin,m]
        ).astype(wnp)
        in_maps.append(
            {
                "y": y_dev,
                "w": w.reshape(IC, OCL, P, NBLK * 4 * P),
                "cf": np.ascontiguousarray(cf[:, osl]).astype(ml_dtypes.bfloat16),
                "bias": np.ascontiguousarray(bias_dev[osl]),
                "ident": ident,
            }
        )
    return in_maps


def _emulate_core(im):
    """Numpy emulation of the device program (for validating layout math)."""
    y = im["y"].astype(np.float32)      # (IC, 128, 512) free=(cp,b)
    w = im["w"].astype(np.float32).reshape(IC, OCL, P, NBLK, 2, 2, P)
    cf = im["cf"].astype(np.float32).reshape(IC, OCL, P, 4, 8)
    out = np.array(im["bias"], dtype=np.float32).reshape(OCL, P, B, NCH).copy()
    bf = lambda a: a.astype(ml_dtypes.bfloat16).astype(np.float32)
    for o in range(OCL):
        acc = np.zeros((P, B, NCH), dtype=np.float32)
        for ic in range(IC):
            yv = y[ic].reshape(P, NCH, B)
            z = np.zeros((P, NCH, B), dtype=np.float32)
            for g in range(NBLK):
                for h in range(2):
                    a = np.zeros((P, B), dtype=np.float32)
                    for kin in range(2):
                        lhsT = w[ic, o, :, g, h, kin]  # [k, m]
                        a += lhsT.T @ yv[:, 2 * g + kin]
                    z[:, 2 * g + h] = a
            # Act transpose-convert -> zb[p, b, n9, n8, n7] bf16
            zb = bf(z.reshape(P, 2, 2, 2, B).transpose(0, 4, 1, 2, 3))
            c = cf[ic, o].reshape(P, 1, 4, 2, 2, 2)
            c8_0 = c[:, :, 0].transpose(0, 1, 3, 2, 4)  # (n8',n9,n7)->(n9,n8',n7)
            c8_1 = c[:, :, 1].transpose(0, 1, 3, 2, 4)
            u0 = bf(zb[:, :, :, 0:1, :] * c8_0)
            u1 = bf(zb[:, :, :, 1:2, :] * c8_1)
            y8 = bf(u0 + u1)                          # [p, b, n9, n8', n7]
            v0 = bf(y8[:, :, 0:1] * c[:, :, 2])
            v1 = bf(y8[:, :, 1:2] * c[:, :, 3])
            acc += (v0 + v1).reshape(P, B, NCH)
        out[o] += acc
    return out.reshape(OCL, P, NCH * B)


def _build_program():
    import concourse.bacc as bacc
    import concourse.mybir as mybir
    from concourse.tile import TileContext

    f32 = mybir.dt.float32
    bf16 = mybir.dt.bfloat16
    _MDT = {"fp8": mybir.dt.float8e4, "fp8e3": mybir.dt.float8e3,
            "bf16": mybir.dt.bfloat16}
    wdt = _MDT[W_DT]
    ydt = _MDT[Y_DT]
    MULT, ADD = mybir.AluOpType.mult, mybir.AluOpType.add
    COPY = mybir.ActivationFunctionType.Copy

    nc = bacc.Bacc(None, target_bir_lowering=False)
    y_d = nc.dram_tensor("y", (IC, P, NCH * B), ydt, kind="ExternalInput")
    w_d = nc.dram_tensor("w", (IC, OCL, P, NBLK * 4 * P), wdt, kind="ExternalInput")
    cf_d = nc.dram_tensor("cf", (IC, OCL, P, 32), bf16, kind="ExternalInput")
    bias_d = nc.dram_tensor("bias", (OCL, P, NCH * B), f32, kind="ExternalInput")
    id_d = nc.dram_tensor("ident", (P, P), bf16, kind="ExternalInput")
    o_d = nc.dram_tensor("o", (OCL, P, NCH * B), f32, kind="ExternalOutput")

    with TileContext(nc) as tc:
        with (
            tc.tile_pool(name="ypool", bufs=2) as ypool,
            tc.tile_pool(name="wpool", bufs=3) as wpool,
            tc.tile_pool(name="cfpool", bufs=3) as cfpool,
            tc.tile_pool(name="zbpool", bufs=2) as zbpool,
            tc.tile_pool(name="upool", bufs=2) as upool,
            tc.tile_pool(name="vpool", bufs=3) as vpool,
            tc.tile_pool(name="misc", bufs=1) as misc,
            tc.tile_pool(name="zpsum", bufs=3, space="PSUM") as zpsum,
            tc.tile_pool(name="apsum", bufs=OCL, space="PSUM") as apsum,
        ):
            ident = misc.tile([P, P], bf16, tag="ident")
            nc.sync.dma_start(out=ident[:], in_=id_d[:, :])
            accs = []
            for o in range(OCL):
                acc = apsum.tile([P, NCH * B], f32, tag="acc")
                accs.append(acc)

            # Software pipelining, two deferred stages:
            #   v-products of pair i emit during pair i+1 (so DVE's in-order
            #   stream never head-of-line blocks on the Pool y8-add), and
            #   acc-matmuls of pair i emit during pair i+2 (so PE's in-order
            #   stream never waits on DVE — by then the v tiles are ready).
            pend_v = None  # (o, y8, cg, first, last)
            accq = []      # [(o, [v0, v1], first, last)] awaiting acc-matmuls

            def emit_acc(entry):
                o, vts, first, last = entry
                for q, vt in enumerate(vts):
                    nc.tensor.matmul(
                        accs[o][:],
                        ident[:],
                        vt[:].rearrange("p b x y z -> p (b x y z)"),
                        start=(first and q == 0), stop=(last and q == 1),
                        skip_group_check=True,
                    )

            def flush_acc(keep):
                while len(accq) > keep:
                    emit_acc(accq.pop(0))

            def flush_v():
                nonlocal pend_v
                if pend_v is None:
                    return
                o, y8, cg, first, last = pend_v
                vts = []
                for q in range(2):
                    vt = vpool.tile([P, B, 2, 2, 2], bf16, tag=f"v{q}",
                                    name=f"v{q}")
                    # v_q[p,b,n9o,n8o,n7] = y8[p,b,q,n8o,n7] * c9q[p,n9o,n8o,n7]
                    yg = y8[:, :, q, :, :]                  # [P,B,2,2]
                    for n9o in range(2):
                        cq = cg[:, 2 + q, n9o].unsqueeze(1).broadcast_to(
                            (P, B, 2, 2)
                        )
                        nc.vector.tensor_tensor(
                            vt[:, :, n9o, :, :], yg, cq, MULT
                        )
                    vts.append(vt)
                accq.append((o, vts, first, last))
                pend_v = None

            for ic in range(IC):
                ytile = ypool.tile([P, NCH * B], ydt)
                nc.sync.dma_start(out=ytile[:], in_=y_d[ic])
                for o in range(OCL):
                    wtile = wpool.tile([P, NBLK * 4 * P], wdt)
                    nc.sync.dma_start(out=wtile[:], in_=w_d[ic, o])
                    cft = cfpool.tile([P, 4, 8], bf16)
                    nc.sync.dma_start(
                        out=cft[:], in_=cf_d[ic, o].rearrange("p (j c) -> p j c", j=4)
                    )
                    z = zpsum.tile([P, NCH * B], f32)
                    for g in range(NBLK):
                        for h in range(2):
                            cp = 2 * g + h
                            for kin in range(2):
                                wi = ((g * 2 + h) * 2 + kin) * P
                                nc.tensor.matmul(
                                    z[:, cp * B : (cp + 1) * B],
                                    wtile[:, wi : wi + P],
                                    ytile[:, (2 * g + kin) * B : (2 * g + kin + 1) * B],
                                    start=(kin == 0),
                                    stop=(kin == 1),
                                )
                    # acc-matmuls of pair i-2 (v tiles long since ready)
                    flush_acc(keep=1)
                    # Act: transpose-convert z (cp,b) f32 -> zb (b,n9,n8,n7) bf16.
                    # Strided INPUT AP (free on Act), contiguous packed output.
                    zb = zbpool.tile([P, B, 2, 2, 2], bf16)
                    z_bc = z[:].rearrange("p (c b) -> p b c", c=NCH)
                    nc.scalar.activation(
                        zb[:].rearrange("p b x y z -> p b (x y z)"),
                        z_bc, COPY, scale=1.0,
                    )

                    # Stage B: tensor_tensor products on DVE (2x bf16 mode on
                    # HW; stt does NOT accelerate). HW AP limit is 2 free dims
                    # per operand, so each product op splits over the bit that
                    # is broadcast in its gather operand (2 ops of 256 els).
                    # The u0+u1 add runs on the otherwise-idle Pool engine.
                    cg = cft[:].rearrange("p j (x y z) -> p j x y z", x=2, y=2)
                    u0 = upool.tile([P, B, 2, 2, 2], bf16, tag="u0")
                    u1 = upool.tile([P, B, 2, 2, 2], bf16, tag="u1")
                    for q, ut in ((0, u0), (1, u1)):
                        # u_q[p,b,n9,n8o,n7] = zb[p,b,n9,q,n7] * c8q[p,n8o,n9,n7]
                        zg = zb[:, :, :, q, :]                  # [P,B,2,2]
                        for n8o in range(2):
                            cq = cg[:, q, n8o].unsqueeze(1).broadcast_to(
                                (P, B, 2, 2)
                            )
                            nc.vector.tensor_tensor(
                                ut[:, :, :, n8o, :], zg, cq, MULT
                            )
                    y8 = upool.tile([P, B, 2, 2, 2], bf16, tag="y8")
                    nc.gpsimd.tensor_tensor(
                        y8[:].rearrange("p b x y z -> p (b x y z)"),
                        u0[:].rearrange("p b x y z -> p (b x y z)"),
                        u1[:].rearrange("p b x y z -> p (b x y z)"),
                        ADD,
                    )
                    # v-products of the PREVIOUS pair
                    flush_v()
                    pend_v = (o, y8, cg, ic == 0, ic == IC - 1)
            flush_v()
            flush_acc(keep=0)

            for o in range(OCL):
                biast = misc.tile([P, NCH * B], f32, tag=f"bias{o}")
                nc.sync.dma_start(out=biast[:], in_=bias_d[o])
                outt = misc.tile([P, NCH * B], f32, tag=f"out{o}")
                nc.vector.scalar_tensor_tensor(
                    outt[:], accs[o][:], 1.0, biast[:], MULT, ADD
                )
                nc.sync.dma_start(out=o_d[o], in_=outt[:])
    nc.finalize()
    return nc


_LAST_RESULTS = {"exec_time_ns": None}


def kernel(x, twiddle, bias, _trace=False, _emulate=False):
    in_maps = _prep_host(np.asarray(x), np.asarray(twiddle), np.asarray(bias))
    if _emulate:
        outs = [_emulate_core(im) for im in in_maps]
    else:
        from concourse.bass_utils import run_bass_kernel_spmd

        nc = _build_program()
        res = run_bass_kernel_spmd(nc, in_maps, list(range(NCORES)), trace=_trace)
        _LAST_RESULTS["exec_time_ns"] = res.exec_time_ns
        _LAST_RESULTS["mean_exec_time_ns"] = res.mean_exec_time_ns
        outs = [r["o"] for r in res.results]
    # o[oc_l, p, b*8+cp] -> (OC, B, N) with n = cp*128+p; final (B,OC,H,W)
    # is a pure reinterpret of (OC,B,N) bytes (reference uses .reshape).
    full = np.concatenate(
        [
            np.asarray(o, dtype=np.float32)
            .reshape(OCL, P, B, NCH)
            .transpose(0, 2, 3, 1)
            .reshape(OCL, B, N)
            for o in outs
        ],
        axis=0,
    )
    return np.ascontiguousarray(full).reshape(B, OC, H, W).astype(np.float32)


# revision 26
# speedup vs baseline: 2.8933x; 1.0430x over previous
"""Butterfly-Conv2d (nn_BConv2d) Trainium2 kernel — v2 (low-precision, PE-accumulated).

Math (reference): x(B=64,IC=16,32,32) -> y=x.reshape(IC,B,N=1024)[:,:,bitrev];
broadcast over OC=32; 10 radix-2 butterfly layers with per-(ic,oc) twiddles;
mean over ic; + bias -> (B,OC,32,32).

Strategy (per core: all 16 ic x 4 oc, core-local ic-mean, no collective):
  * Host (free): compose butterfly layers 0..7 into dense 256x256 blocks
    (4 per (ic,oc)); cast weights to fp8/bf16. Build per-(p,chunk) coeff
    vectors for layers 8/9 (1/16 mean folded into layer 9).
  * Stage A (PE): 16 matmuls per (ic,oc) [k=128, m=128, free=64] with PSUM
    K-accumulation -> z[p, (cp,b)] f32 in PSUM.
  * Act: transpose-convert z -> SBUF bf16 in (b, n9, n8, n7) free layout
    (coefficient broadcasts then have packed last dims -> DVE 4x mode).
  * Stage B (DVE): 5 wide scalar_tensor_tensor ops per (ic,oc):
      u_q = z[b, n9, q, n7] * c8_q[n9, n8', n7]   (q=0,1; broadcast over b)
      y8  = u0 + u1
      v_q = y8[b, q, n8', n7] * c9_q[n9', n8', n7]
    (stt with immediate scalar 1.0 => InstTensorScalarPtr, 4x perf mode)
  * ic-mean accumulation (PE): acc_o += I @ v_q, 32 accumulating matmuls
    into a persistent PSUM bank per oc.
  * Epilogue: out_o = acc_o + bias (DVE), DMA out.

Device output layout: o[ocl, p, b*8+cp] with n = cp*128+p.
"""

import numpy as np
import ml_dtypes

B, IC, OC, H, W = 64, 16, 32, 32, 32
N = H * W          # 1024
M = 10             # butterfly layers
NCORES = 8
OCL = OC // NCORES  # 4 oc per core
NCH = 8            # free-dim chunks (n9n8n7)
P = 128            # partitions (n6..n0)
SB = 256           # composed stage-A block size (layers 0..7)
NBLK = N // SB     # 4 blocks per (ic,oc)

# stage-A weight dtype: "fp8e3" (float8_e3m4, per-pair scaled), "fp8"
# (float8_e4m3) or "bf16"
W_DT = "fp8e3"
Y_DT = "bf16"      # stage-A rhs dtype (mixed fp8 lhsT x bf16 rhs verified on HW)

_NPDT = {
    "fp8": ml_dtypes.float8_e4m3,
    "fp8e3": ml_dtypes.float8_e3m4,
    "bf16": ml_dtypes.bfloat16,
}


def _bitrev(n):
    bits = int(np.log2(n))
    idx = np.arange(n, dtype=np.int64)
    rev = np.zeros(n, dtype=np.int64)
    for b in range(bits):
        rev = (rev << 1) | ((idx >> b) & 1)
    return rev


def _compose_stageA(tw):
    """Compose butterfly layers 0..7 into A[ic,oc,g,256,256] (g=4 blocks)."""
    ic, oc = tw.shape[0], tw.shape[1]
    A = np.zeros((ic, oc, NBLK, SB, SB), dtype=np.float32)
    eye = np.eye(SB, dtype=np.float32)
    A[:] = eye
    for l in range(8):
        s = 1 << l
        nb_loc = SB // (2 * s)
        t = tw[:, :, l].reshape(ic, oc, N // (2 * s), s, 2, 2)
        t = t.reshape(ic, oc, NBLK, nb_loc, s, 2, 2)
        Av = A.reshape(ic, oc, NBLK, nb_loc, 2, s, SB)
        a0 = Av[:, :, :, :, 0]
        a1 = Av[:, :, :, :, 1]
        t00 = t[..., 0, 0, None]
        t01 = t[..., 0, 1, None]
        t10 = t[..., 1, 0, None]
        t11 = t[..., 1, 1, None]
        new0 = t00 * a0 + t01 * a1
        new1 = t10 * a0 + t11 * a1
        Av[:, :, :, :, 0] = new0
        Av[:, :, :, :, 1] = new1
    return A


def _stageB_coeffs(tw):
    """cf[ic, oc, p, 32] f32: 4 groups of 8 chunk-coeffs per partition.

    group j=0/1: layer-8 q=0/1 coeff, stored n8'-major (n8', n9, n7) so the
        per-n8' slice is contiguous (HW AP limit: 2 free dims/operand):
        cf[.., q, n8o*4+n9*2+n7] = t8[k=n9, n7*128+p, n8', q]
    group j=2/3: layer-9 q9=0/1 coeff, stored n9'-major (n9', n8', n7):
        cf[.., 2+q, n9o*4+n8o*2+n7] = t9[0, n8'*256+n7*128+p, n9', q9] / IC
    """
    ic, oc = tw.shape[0], tw.shape[1]
    t8 = tw[:, :, 8].reshape(ic, oc, 2, 256, 2, 2)   # [k, j, p_out, q]
    t9 = tw[:, :, 9].reshape(ic, oc, 512, 2, 2)      # [j, p_out, q]
    cf = np.zeros((ic, oc, P, 4, 8), dtype=np.float32)
    pr = np.arange(P)
    for n9 in range(2):
        for n8o in range(2):
            for n7 in range(2):
                for q in range(2):
                    cf[:, :, :, q, n8o * 4 + n9 * 2 + n7] = t8[
                        :, :, n9, n7 * 128 + pr, n8o, q
                    ]
                    cf[:, :, :, 2 + q, n9 * 4 + n8o * 2 + n7] = (
                        t9[:, :, n8o * 256 + n7 * 128 + pr, n9, q] / IC
                    )
    return cf.reshape(ic, oc, P, 32)


def _prep_host(x, twiddle, bias):
    """All host-side layout work. Returns per-core input maps (numpy)."""
    wnp = _NPDT[W_DT]
    ynp = _NPDT[Y_DT]
    perm = _bitrev(N)
    y = np.ascontiguousarray(x).reshape(IC, B, N)[:, :, perm]
    # device layout y[ic, p, c*64+b]
    y_dev = np.ascontiguousarray(
        y.reshape(IC, B, NCH, P).transpose(0, 3, 2, 1)
    ).reshape(IC, P, NCH * B).astype(ynp)

    A = _compose_stageA(np.asarray(twiddle, dtype=np.float32))
    cf = _stageB_coeffs(np.asarray(twiddle, dtype=np.float32))
    if W_DT == "fp8e3":
        # per-(ic,oc) scale: use e3m4's range, fold 1/s into layer-8 coeffs
        amax = np.abs(A).max(axis=(2, 3, 4))          # (IC, OC)
        s = 8.0 / np.maximum(amax, 1e-30)
        A = A * s[:, :, None, None, None]
        cfv = cf.reshape(IC, OC, P, 4, 8)
        cfv[:, :, :, 0:2] /= s[:, :, None, None, None]

    # bias in device (b,cp) layout: bias_dev[oc, p, b*8+cp] = bias[oc, cp*128+p]
    bias_pc = np.asarray(bias, dtype=np.float32).reshape(OC, NCH, P)
    bias_dev = np.broadcast_to(
        bias_pc.transpose(0, 2, 1)[:, :, None, :], (OC, P, B, NCH)
    ).reshape(OC, P, NCH * B)

    ident = np.eye(P, dtype=np.float32).astype(ml_dtypes.bfloat16)

    in_maps = []
    for core in range(NCORES):
        osl = slice(core * OCL, (core + 1) * OCL)
        Ac = A[:, osl]  # (IC, OCL, 4, 256, 256)
        # lhsT tiles: w[ic,o,p_k, g, h, kin, m] = Ac[ic,o,g][h*128+m, kin*128+p_k]
        w = np.ascontiguousarray(
            Ac.reshape(IC, OCL, NBLK, 2, P, 2, P)  # [g, h, m, kin, k]
            .transpose(0, 1, 6, 2, 3, 5, 4)        # [ic,o,k,g,h,kin,m]
        ).astype(wnp)
        in_maps.append(
            {
                "y": y_dev,
                "w": w.reshape(IC, OCL, P, NBLK * 4 * P),
                "cf": np.ascontiguousarray(cf[:, osl]).astype(ml_dtypes.bfloat16),
                "bias": np.ascontiguousarray(bias_dev[osl]),
                "ident": ident,
            }
        )
    return in_maps


def _emulate_core(im):
    """Numpy emulation of the device program (for validating layout math)."""
    y = im["y"].astype(np.float32)      # (IC, 128, 512) free=(cp,b)
    w = im["w"].astype(np.float32).reshape(IC, OCL, P, NBLK, 2, 2, P)
    cf = im["cf"].astype(np.float32).reshape(IC, OCL, P, 4, 8)
    out = np.array(im["bias"], dtype=np.float32).reshape(OCL, P, B, NCH).copy()
    bf = lambda a: a.astype(ml_dtypes.bfloat16).astype(np.float32)
    for o in range(OCL):
        acc = np.zeros((P, B, NCH), dtype=np.float32)
        for ic in range(IC):
            yv = y[ic].reshape(P, NCH, B)
            z = np.zeros((P, NCH, B), dtype=np.float32)
            for g in range(NBLK):
                for h in range(2):
                    a = np.zeros((P, B), dtype=np.float32)
                    for kin in range(2):
                        lhsT = w[ic, o, :, g, h, kin]  # [k, m]
                        a += lhsT.T @ yv[:, 2 * g + kin]
                    z[:, 2 * g + h] = a
            # Act transpose-convert -> zb[p, b, n9, n8, n7] bf16
            zb = bf(z.reshape(P, 2, 2, 2, B).transpose(0, 4, 1, 2, 3))
            c = cf[ic, o].reshape(P, 1, 4, 2, 2, 2)
            c8_0 = c[:, :, 0].transpose(0, 1, 3, 2, 4)  # (n8',n9,n7)->(n9,n8',n7)
            c8_1 = c[:, :, 1].transpose(0, 1, 3, 2, 4)
            u0 = bf(zb[:, :, :, 0:1, :] * c8_0)
            u1 = bf(zb[:, :, :, 1:2, :] * c8_1)
            y8 = bf(u0 + u1)                          # [p, b, n9, n8', n7]
            v0 = bf(y8[:, :, 0:1] * c[:, :, 2])
            v1 = bf(y8[:, :, 1:2] * c[:, :, 3])
            acc += (v0 + v1).reshape(P, B, NCH)
        out[o] += acc
    return out.reshape(OCL, P, NCH * B)


def _build_program():
    import concourse.bacc as bacc
    import concourse.mybir as mybir
    from concourse.tile import TileContext

    f32 = mybir.dt.float32
    bf16 = mybir.dt.bfloat16
    _MDT = {"fp8": mybir.dt.float8e4, "fp8e3": mybir.dt.float8e3,
            "bf16": mybir.dt.bfloat16}
    wdt = _MDT[W_DT]
    ydt = _MDT[Y_DT]
    MULT, ADD = mybir.AluOpType.mult, mybir.AluOpType.add
    COPY = mybir.ActivationFunctionType.Copy

    nc = bacc.Bacc(None, target_bir_lowering=False)
    y_d = nc.dram_tensor("y", (IC, P, NCH * B), ydt, kind="ExternalInput")
    w_d = nc.dram_tensor("w", (IC, OCL, P, NBLK * 4 * P), wdt, kind="ExternalInput")
    cf_d = nc.dram_tensor("cf", (IC, OCL, P, 32), bf16, kind="ExternalInput")
    bias_d = nc.dram_tensor("bias", (OCL, P, NCH * B), f32, kind="ExternalInput")
    id_d = nc.dram_tensor("ident", (P, P), bf16, kind="ExternalInput")
    o_d = nc.dram_tensor("o", (OCL, P, NCH * B), f32, kind="ExternalOutput")

    with TileContext(nc) as tc:
        with (
            tc.tile_pool(name="ypool", bufs=2) as ypool,
            tc.tile_pool(name="wpool", bufs=3) as wpool,
            tc.tile_pool(name="cfpool", bufs=3) as cfpool,
            tc.tile_pool(name="zbpool", bufs=3) as zbpool,
            tc.tile_pool(name="upool", bufs=3) as upool,
            tc.tile_pool(name="vpool", bufs=3) as vpool,
            tc.tile_pool(name="misc", bufs=1) as misc,
            tc.tile_pool(name="zpsum", bufs=3, space="PSUM") as zpsum,
            tc.tile_pool(name="apsum", bufs=OCL, space="PSUM") as apsum,
        ):
            ident = misc.tile([P, P], bf16, tag="ident")
            nc.sync.dma_start(out=ident[:], in_=id_d[:, :])
            accs = []
            for o in range(OCL):
                acc = apsum.tile([P, NCH * B], f32, tag="acc")
                accs.append(acc)

            # Software pipelining, two deferred stages:
            #   v-products of pair i emit during pair i+1 (so DVE's in-order
            #   stream never head-of-line blocks on the Pool y8-add), and
            #   acc-matmuls of pair i emit during pair i+2 (so PE's in-order
            #   stream never waits on DVE — by then the v tiles are ready).
            pend_v = None  # (o, y8, cg, first, last)
            accq = []      # [(o, [v0, v1], first, last)] awaiting acc-matmuls

            def emit_acc(entry):
                o, vts, first, last = entry
                for q, vt in enumerate(vts):
                    nc.tensor.matmul(
                        accs[o][:],
                        ident[:],
                        vt[:].rearrange("p b x y z -> p (b x y z)"),
                        start=(first and q == 0), stop=(last and q == 1),
                        skip_group_check=True,
                    )

            def flush_acc(keep):
                while len(accq) > keep:
                    emit_acc(accq.pop(0))

            def flush_v():
                nonlocal pend_v
                if pend_v is None:
                    return
                o, y8, cg, first, last = pend_v
                vts = []
                for q in range(2):
                    vt = vpool.tile([P, B, 2, 2, 2], bf16, tag=f"v{q}",
                                    name=f"v{q}")
                    # v_q[p,b,n9o,n8o,n7] = y8[p,b,q,n8o,n7] * c9q[p,n9o,n8o,n7]
                    yg = y8[:, :, q, :, :]                  # [P,B,2,2]
                    for n9o in range(2):
                        cq = cg[:, 2 + q, n9o].unsqueeze(1).broadcast_to(
                            (P, B, 2, 2)
                        )
                        nc.vector.tensor_tensor(
                            vt[:, :, n9o, :, :], yg, cq, MULT
                        )
                    vts.append(vt)
                accq.append((o, vts, first, last))
                pend_v = None

            for ic in range(IC):
                ytile = ypool.tile([P, NCH * B], ydt)
                nc.sync.dma_start(out=ytile[:], in_=y_d[ic])
                for o in range(OCL):
                    wtile = wpool.tile([P, NBLK * 4 * P], wdt)
                    nc.sync.dma_start(out=wtile[:], in_=w_d[ic, o])
                    cft = cfpool.tile([P, 4, 8], bf16)
                    nc.sync.dma_start(
                        out=cft[:], in_=cf_d[ic, o].rearrange("p (j c) -> p j c", j=4)
                    )
                    z = zpsum.tile([P, NCH * B], f32)
                    for g in range(NBLK):
                        for h in range(2):
                            cp = 2 * g + h
                            for kin in range(2):
                                wi = ((g * 2 + h) * 2 + kin) * P
                                nc.tensor.matmul(
                                    z[:, cp * B : (cp + 1) * B],
                                    wtile[:, wi : wi + P],
                                    ytile[:, (2 * g + kin) * B : (2 * g + kin + 1) * B],
                                    start=(kin == 0),
                                    stop=(kin == 1),
                                )
                    # acc-matmuls of pair i-2 (v tiles long since ready)
                    flush_acc(keep=1)
                    # Act: transpose-convert z (cp,b) f32 -> zb (b,n9,n8,n7) bf16.
                    # Strided INPUT AP (free on Act), contiguous packed output.
                    zb = zbpool.tile([P, B, 2, 2, 2], bf16)
                    z_bc = z[:].rearrange("p (c b) -> p b c", c=NCH)
                    nc.scalar.activation(
                        zb[:].rearrange("p b x y z -> p b (x y z)"),
                        z_bc, COPY, scale=1.0,
                    )

                    # Stage B: tensor_tensor products on DVE (2x bf16 mode on
                    # HW; stt does NOT accelerate). HW AP limit is 2 free dims
                    # per operand, so each product op splits over the bit that
                    # is broadcast in its gather operand (2 ops of 256 els).
                    # The u0+u1 add runs on the otherwise-idle Pool engine.
                    cg = cft[:].rearrange("p j (x y z) -> p j x y z", x=2, y=2)
                    u0 = upool.tile([P, B, 2, 2, 2], bf16, tag="u0")
                    u1 = upool.tile([P, B, 2, 2, 2], bf16, tag="u1")
                    for q, ut in ((0, u0), (1, u1)):
                        # u_q[p,b,n9,n8o,n7] = zb[p,b,n9,q,n7] * c8q[p,n8o,n9,n7]
                        zg = zb[:, :, :, q, :]                  # [P,B,2,2]
                        for n8o in range(2):
                            cq = cg[:, q, n8o].unsqueeze(1).broadcast_to(
                                (P, B, 2, 2)
                            )
                            nc.vector.tensor_tensor(
                                ut[:, :, :, n8o, :], zg, cq, MULT
                            )
                    y8 = upool.tile([P, B, 2, 2, 2], bf16, tag="y8")
                    nc.vector.tensor_tensor(
                        y8[:].rearrange("p b x y z -> p (b x y z)"),
                        u0[:].rearrange("p b x y z -> p (b x y z)"),
                        u1[:].rearrange("p b x y z -> p (b x y z)"),
                        ADD,
                    )
                    # v-products of the PREVIOUS pair
                    flush_v()
                    pend_v = (o, y8, cg, ic == 0, ic == IC - 1)
            flush_v()
            flush_acc(keep=0)

            for o in range(OCL):
                biast = misc.tile([P, NCH * B], f32, tag=f"bias{o}")
                nc.sync.dma_start(out=biast[:], in_=bias_d[o])
                outt = misc.tile([P, NCH * B], f32, tag=f"out{o}")
                nc.vector.scalar_tensor_tensor(
                    outt[:], accs[o][:], 1.0, biast[:], MULT, ADD
                )
                nc.sync.dma_start(out=o_d[o], in_=outt[:])
    nc.finalize()
    return nc


_LAST_RESULTS = {"exec_time_ns": None}


def kernel(x, twiddle, bias, _trace=False, _emulate=False):
    in_maps = _prep_host(np.asarray(x), np.asarray(twiddle), np.asarray(bias))
    if _emulate:
        outs = [_emulate_core(im) for im in in_maps]
    else:
        from concourse.bass_utils import run_bass_kernel_spmd

        nc = _build_program()
        res = run_bass_kernel_spmd(nc, in_maps, list(range(NCORES)), trace=_trace)
        _LAST_RESULTS["exec_time_ns"] = res.exec_time_ns
        _LAST_RESULTS["mean_exec_time_ns"] = res.mean_exec_time_ns
        outs = [r["o"] for r in res.results]
    # o[oc_l, p, b*8+cp] -> (OC, B, N) with n = cp*128+p; final (B,OC,H,W)
    # is a pure reinterpret of (OC,B,N) bytes (reference uses .reshape).
    full = np.concatenate(
        [
            np.asarray(o, dtype=np.float32)
            .reshape(OCL, P, B, NCH)
            .transpose(0, 2, 3, 1)
            .reshape(OCL, B, N)
            for o in outs
        ],
        axis=0,
    )
    return np.ascontiguousarray(full).reshape(B, OC, H, W).astype(np.float32)


# revision 27
# speedup vs baseline: 3.1767x; 1.0979x over previous
"""Butterfly-Conv2d (nn_BConv2d) Trainium2 kernel — v7 (layer-8 folded into PE).

Math (reference): x(B=64,IC=16,32,32) -> y=x.reshape(IC,B,N=1024)[:,:,bitrev];
broadcast over OC=32; 10 radix-2 butterfly layers with per-(ic,oc) twiddles;
mean over ic; + bias -> (B,OC,32,32).

Strategy (per core: all 16 ic x 4 oc, core-local ic-mean, no collective):
  * Host (free): compose butterfly layers 0..7 into dense 256x256 blocks;
    FOLD layer 8 into the matmul weights: each out chunk cp=(n9,n8',n7) is
      y8[p,cp] = sum_q t8_q[p,cp] * z[p,src_q(cp)]
    i.e. 4 PSUM-accumulated matmuls per out chunk whose lhsT columns are
    scaled by t8_q[m,cp]. Weight elements double (32 blocks of 128x128 per
    (ic,oc)) but stage B shrinks to just layer 9.
  * Weights in float8_e3m4 with a per-(pair,chunk) scale (max -> 8.0); the
    inverse scale is folded into the layer-9 coefficients.
  * PE: 32 matmuls per (ic,oc) -> y8[p,(cp,b)] f32 in PSUM (start only on
    each chunk's first MM: PSUM 'start' zeroes the addressed region).
  * Act: strided-input transpose-convert y8 -> SBUF bf16 (b,n9,n8,n7).
  * DVE: 4 tensor_tensor product ops per (ic,oc) (layer 9, 2x bf16 mode):
      v_q[p,b,n9o,n8o,n7] = y8b[p,b,q,n8o,n7] * c9q[p,n9o,n8o,n7]
  * ic-mean (PE): acc_o += I @ v_q, accumulating identity matmuls into a
    persistent PSUM bank per oc (deferred 1 pair so PE never waits on DVE).
  * Epilogue: out_o = acc_o + bias (DVE), DMA out.

Device output layout: o[ocl, p, b*8+cp] with n = cp*128+p.
"""

import numpy as np
import ml_dtypes

B, IC, OC, H, W = 64, 16, 32, 32, 32
N = H * W          # 1024
M = 10             # butterfly layers
NCORES = 8
OCL = OC // NCORES  # 4 oc per core
NCH = 8            # free-dim chunks (n9n8n7)
P = 128            # partitions (n6..n0)
SB = 256           # composed stage-A block size (layers 0..7)
NBLK = N // SB     # 4 blocks per (ic,oc)
NMM = 32           # folded matmuls per (ic,oc): 8 chunks x (2 q x 2 kin)

# folded stage-A weight dtype: "fp8e3" (float8_e3m4, per-chunk scaled),
# "fp8" (float8_e4m3) or "bf16"
W_DT = "fp8e3"
Y_DT = "bf16"

_NPDT = {
    "fp8": ml_dtypes.float8_e4m3,
    "fp8e3": ml_dtypes.float8_e3m4,
    "bf16": ml_dtypes.bfloat16,
}


def _bitrev(n):
    bits = int(np.log2(n))
    idx = np.arange(n, dtype=np.int64)
    rev = np.zeros(n, dtype=np.int64)
    for b in range(bits):
        rev = (rev << 1) | ((idx >> b) & 1)
    return rev


def _compose_stageA(tw):
    """Compose butterfly layers 0..7 into A[ic,oc,g,256,256] (g=4 blocks)."""
    ic, oc = tw.shape[0], tw.shape[1]
    A = np.zeros((ic, oc, NBLK, SB, SB), dtype=np.float32)
    eye = np.eye(SB, dtype=np.float32)
    A[:] = eye
    for l in range(8):
        s = 1 << l
        nb_loc = SB // (2 * s)
        t = tw[:, :, l].reshape(ic, oc, N // (2 * s), s, 2, 2)
        t = t.reshape(ic, oc, NBLK, nb_loc, s, 2, 2)
        Av = A.reshape(ic, oc, NBLK, nb_loc, 2, s, SB)
        a0 = Av[:, :, :, :, 0]
        a1 = Av[:, :, :, :, 1]
        t00 = t[..., 0, 0, None]
        t01 = t[..., 0, 1, None]
        t10 = t[..., 1, 0, None]
        t11 = t[..., 1, 1, None]
        new0 = t00 * a0 + t01 * a1
        new1 = t10 * a0 + t11 * a1
        Av[:, :, :, :, 0] = new0
        Av[:, :, :, :, 1] = new1
    return A


def _fold_weights(tw, A):
    """Fold layer 8 into stage-A weights + build layer-9 coeffs.

    Returns:
      w2[ic, oc, 128(k), 32(cp,q,kin), 128(m)] f32 — lhsT blocks, columns m
        scaled by t8_q[m, cp], per-(ic,oc,cp) rescaled for W_DT range;
      cf9[ic, oc, P, 2(q9), 8(n9o,n8o,n7)] f32 — layer-9 coeffs (1/IC and
        the inverse weight scales folded in).
    """
    ic, oc = tw.shape[0], tw.shape[1]
    t8 = tw[:, :, 8].reshape(ic, oc, 2, 256, 2, 2)   # [k, j, n8', q]
    t9 = tw[:, :, 9].reshape(ic, oc, 512, 2, 2)      # [j, n9', q9]
    pr = np.arange(P)

    # t8c[ic, oc, m(=p), cp, q]: scale for out chunk cp=(n9,n8o,n7)
    t8c = np.zeros((ic, oc, P, NCH, 2), dtype=np.float32)
    for cp in range(NCH):
        n9, n8o, n7 = cp >> 2, (cp >> 1) & 1, cp & 1
        for q in range(2):
            t8c[:, :, :, cp, q] = t8[:, :, n9, n7 * 128 + pr, n8o, q]

    # base lhsT blocks: wb[ic,oc,k, g, h, kin, m] = A[g][h*128+m, kin*128+k]
    wb = A.reshape(ic, oc, NBLK, 2, P, 2, P).transpose(0, 1, 6, 2, 3, 5, 4)
    # w2[ic,oc,k, cp, q, kin, m] = wb[.., g'(cp,q), h'(cp), kin, m]*t8c[m,cp,q]
    w2 = np.zeros((ic, oc, P, NCH, 2, 2, P), dtype=np.float32)
    for cp in range(NCH):
        n9, n8o, n7 = cp >> 2, (cp >> 1) & 1, cp & 1
        for q in range(2):
            gp, hp = n9 * 2 + q, n7
            w2[:, :, :, cp, q] = (
                wb[:, :, :, gp, hp] * t8c[:, :, None, None, :, cp, q]
            )

    # layer-9 coeffs, n9o-major storage: cf9[.., q9, n9o*4+n8o*2+n7]
    cf9 = np.zeros((ic, oc, P, 2, NCH), dtype=np.float32)
    for n9 in range(2):
        for n8o in range(2):
            for n7 in range(2):
                for q in range(2):
                    cf9[:, :, :, q, n9 * 4 + n8o * 2 + n7] = (
                        t9[:, :, n8o * 256 + n7 * 128 + pr, n9, q] / IC
                    )

    if W_DT == "fp8e3":
        # per-(ic,oc,cp) scale (shared by the chunk's 4 accumulated MMs)
        amax = np.abs(w2).max(axis=(2, 4, 5, 6))      # (ic, oc, NCH)
        s = 8.0 / np.maximum(amax, 1e-30)
        w2 *= s[:, :, None, :, None, None, None]
        # v_q9 sources y8 chunk src=(q9, n8o, n7) -> unscale by 1/s[src]
        for cpo in range(NCH):
            n9o, n8o, n7 = cpo >> 2, (cpo >> 1) & 1, cpo & 1
            for q9 in range(2):
                src = q9 * 4 + n8o * 2 + n7
                cf9[:, :, :, q9, cpo] /= s[:, :, None, src]
    return w2, cf9


def _prep_host(x, twiddle, bias):
    """All host-side layout work. Returns per-core input maps (numpy)."""
    wnp = _NPDT[W_DT]
    ynp = _NPDT[Y_DT]
    perm = _bitrev(N)
    y = np.ascontiguousarray(x).reshape(IC, B, N)[:, :, perm]
    y_dev = np.ascontiguousarray(
        y.reshape(IC, B, NCH, P).transpose(0, 3, 2, 1)
    ).reshape(IC, P, NCH * B).astype(ynp)

    tw = np.asarray(twiddle, dtype=np.float32)
    A = _compose_stageA(tw)
    w2, cf9 = _fold_weights(tw, A)

    bias_pc = np.asarray(bias, dtype=np.float32).reshape(OC, NCH, P)
    bias_dev = np.broadcast_to(
        bias_pc.transpose(0, 2, 1)[:, :, None, :], (OC, P, B, NCH)
    ).reshape(OC, P, NCH * B)

    ident = np.eye(P, dtype=np.float32).astype(ml_dtypes.bfloat16)

    in_maps = []
    for core in range(NCORES):
        osl = slice(core * OCL, (core + 1) * OCL)
        in_maps.append(
            {
                "y": y_dev,
                "w": np.ascontiguousarray(w2[:, osl]).astype(wnp).reshape(
                    IC, OCL, P, NMM * P
                ),
                "cf": np.ascontiguousarray(cf9[:, osl]).astype(
                    ml_dtypes.bfloat16
                ).reshape(IC, OCL, P, 16),
                "bias": np.ascontiguousarray(bias_dev[osl]),
                "ident": ident,
            }
        )
    return in_maps


def _emulate_core(im):
    """Numpy emulation of the device program (for validating layout math)."""
    y = im["y"].astype(np.float32)      # (IC, 128, 512) free=(cp,b)
    w = im["w"].astype(np.float32).reshape(IC, OCL, P, NCH, 2, 2, P)
    cf = im["cf"].astype(np.float32).reshape(IC, OCL, P, 2, NCH)
    out = np.array(im["bias"], dtype=np.float32).reshape(OCL, P, B, NCH).copy()
    bf = lambda a: a.astype(ml_dtypes.bfloat16).astype(np.float32)
    for o in range(OCL):
        acc = np.zeros((P, B, NCH), dtype=np.float32)
        for ic in range(IC):
            yv = y[ic].reshape(P, NCH, B)
            y8 = np.zeros((P, NCH, B), dtype=np.float32)
            for cp in range(NCH):
                n9, q_, n7 = cp >> 2, 0, cp & 1
                a = np.zeros((P, B), dtype=np.float32)
                for q in range(2):
                    gp = (cp >> 2) * 2 + q
                    for kin in range(2):
                        lhsT = w[ic, o, :, cp, q, kin]  # [k, m]
                        a += lhsT.T @ yv[:, 2 * gp + kin]
                y8[:, cp] = a
            # Act transpose-convert -> y8b[p, b, n9, n8, n7] bf16
            y8b = bf(y8.reshape(P, 2, 2, 2, B).transpose(0, 4, 1, 2, 3))
            c = cf[ic, o].reshape(P, 1, 2, 2, 2, 2)  # [p,1,q9,n9o,n8o,n7]
            v0 = bf(y8b[:, :, 0:1] * c[:, :, 0])
            v1 = bf(y8b[:, :, 1:2] * c[:, :, 1])
            acc += (v0 + v1).reshape(P, B, NCH)
        out[o] += acc
    return out.reshape(OCL, P, NCH * B)


def _build_program():
    import concourse.bacc as bacc
    import concourse.mybir as mybir
    from concourse.tile import TileContext

    f32 = mybir.dt.float32
    bf16 = mybir.dt.bfloat16
    _MDT = {"fp8": mybir.dt.float8e4, "fp8e3": mybir.dt.float8e3,
            "bf16": mybir.dt.bfloat16}
    wdt = _MDT[W_DT]
    ydt = _MDT[Y_DT]
    MULT, ADD = mybir.AluOpType.mult, mybir.AluOpType.add
    COPY = mybir.ActivationFunctionType.Copy

    nc = bacc.Bacc(None, target_bir_lowering=False)
    y_d = nc.dram_tensor("y", (IC, P, NCH * B), ydt, kind="ExternalInput")
    w_d = nc.dram_tensor("w", (IC, OCL, P, NMM * P), wdt, kind="ExternalInput")
    cf_d = nc.dram_tensor("cf", (IC, OCL, P, 16), bf16, kind="ExternalInput")
    bias_d = nc.dram_tensor("bias", (OCL, P, NCH * B), f32, kind="ExternalInput")
    id_d = nc.dram_tensor("ident", (P, P), bf16, kind="ExternalInput")
    o_d = nc.dram_tensor("o", (OCL, P, NCH * B), f32, kind="ExternalOutput")

    with TileContext(nc) as tc:
        with (
            tc.tile_pool(name="ypool", bufs=2) as ypool,
            tc.tile_pool(name="wpool", bufs=3) as wpool,
            tc.tile_pool(name="cfpool", bufs=3) as cfpool,
            tc.tile_pool(name="zbpool", bufs=3) as zbpool,
            tc.tile_pool(name="vpool", bufs=3) as vpool,
            tc.tile_pool(name="misc", bufs=1) as misc,
            tc.tile_pool(name="zpsum", bufs=3, space="PSUM") as zpsum,
            tc.tile_pool(name="apsum", bufs=OCL, space="PSUM") as apsum,
        ):
            ident = misc.tile([P, P], bf16, tag="ident")
            nc.sync.dma_start(out=ident[:], in_=id_d[:, :])
            accs = []
            for o in range(OCL):
                acc = apsum.tile([P, NCH * B], f32, tag="acc")
                accs.append(acc)

            accq = []  # [(o, [v0, v1], first, last)] awaiting acc-matmuls

            def emit_acc(entry):
                o, vts, first, last = entry
                for q, vt in enumerate(vts):
                    nc.tensor.matmul(
                        accs[o][:],
                        ident[:],
                        vt[:].rearrange("p b x y z -> p (b x y z)"),
                        start=(first and q == 0), stop=(last and q == 1),
                        skip_group_check=True,
                    )

            def flush_acc(keep):
                while len(accq) > keep:
                    emit_acc(accq.pop(0))

            for ic in range(IC):
                ytile = ypool.tile([P, NCH * B], ydt)
                nc.sync.dma_start(out=ytile[:], in_=y_d[ic])
                for o in range(OCL):
                    wtile = wpool.tile([P, NMM * P], wdt)
                    nc.sync.dma_start(out=wtile[:], in_=w_d[ic, o])
                    cft = cfpool.tile([P, 2, 8], bf16)
                    nc.sync.dma_start(
                        out=cft[:], in_=cf_d[ic, o].rearrange("p (j c) -> p j c", j=2)
                    )
                    z = zpsum.tile([P, NCH * B], f32)
                    for cp in range(NCH):
                        first_mm = True
                        for q in range(2):
                            gp = (cp >> 2) * 2 + q
                            for kin in range(2):
                                wi = ((cp * 2 + q) * 2 + kin) * P
                                nc.tensor.matmul(
                                    z[:, cp * B : (cp + 1) * B],
                                    wtile[:, wi : wi + P],
                                    ytile[:, (2 * gp + kin) * B : (2 * gp + kin + 1) * B],
                                    start=first_mm,
                                    stop=(q == 1 and kin == 1),
                                )
                                first_mm = False
                    # acc-matmuls of the previous pair (v tiles ready)
                    flush_acc(keep=0 if (ic == IC - 1 and o == OCL - 1) else 1)

                    # Act: strided-in transpose-convert y8 (cp,b) f32 ->
                    # zb (b, n9, n8, n7) bf16
                    zb = zbpool.tile([P, B, 2, 2, 2], bf16)
                    z_bc = z[:].rearrange("p (c b) -> p b c", c=NCH)
                    nc.scalar.activation(
                        zb[:].rearrange("p b x y z -> p b (x y z)"),
                        z_bc, COPY, scale=1.0,
                    )

                    # DVE layer 9: 4 tensor_tensor products (2x bf16 mode)
                    cg = cft[:].rearrange("p j (x y z) -> p j x y z", x=2, y=2)
                    vts = []
                    for q in range(2):
                        vt = vpool.tile([P, B, 2, 2, 2], bf16, tag=f"v{q}",
                                        name=f"v{q}")
                        yg = zb[:, :, q, :, :]              # [P,B,2,2]
                        for n9o in range(2):
                            cq = cg[:, q, n9o].unsqueeze(1).broadcast_to(
                                (P, B, 2, 2)
                            )
                            nc.vector.tensor_tensor(
                                vt[:, :, n9o, :, :], yg, cq, MULT
                            )
                        vts.append(vt)
                    accq.append((o, vts, ic == 0, ic == IC - 1))
            flush_acc(keep=0)

            for o in range(OCL):
                biast = misc.tile([P, NCH * B], f32, tag=f"bias{o}")
                nc.sync.dma_start(out=biast[:], in_=bias_d[o])
                outt = misc.tile([P, NCH * B], f32, tag=f"out{o}")
                nc.vector.scalar_tensor_tensor(
                    outt[:], accs[o][:], 1.0, biast[:], MULT, ADD
                )
                nc.sync.dma_start(out=o_d[o], in_=outt[:])
    nc.finalize()
    return nc


_LAST_RESULTS = {"exec_time_ns": None}


def kernel(x, twiddle, bias, _trace=False, _emulate=False):
    in_maps = _prep_host(np.asarray(x), np.asarray(twiddle), np.asarray(bias))
    if _emulate:
        outs = [_emulate_core(im) for im in in_maps]
    else:
        from concourse.bass_utils import run_bass_kernel_spmd

        nc = _build_program()
        res = run_bass_kernel_spmd(nc, in_maps, list(range(NCORES)), trace=_trace)
        _LAST_RESULTS["exec_time_ns"] = res.exec_time_ns
        _LAST_RESULTS["mean_exec_time_ns"] = res.mean_exec_time_ns
        outs = [r["o"] for r in res.results]
    # o[oc_l, p, b*8+cp] -> (OC, B, N) with n = cp*128+p; final (B,OC,H,W)
    # is a pure reinterpret of (OC,B,N) bytes (reference uses .reshape).
    full = np.concatenate(
        [
            np.asarray(o, dtype=np.float32)
            .reshape(OCL, P, B, NCH)
            .transpose(0, 2, 3, 1)
            .reshape(OCL, B, N)
            for o in outs
        ],
        axis=0,
    )
    return np.ascontiguousarray(full).reshape(B, OC, H, W).astype(np.float32)
